# revision 12
# baseline (speedup 1.0000x reference)
"""Bipartite GCN message-passing kernel for 8 Trainium2 NeuronCores.

Math (reference): rst = deg_in^-1/2 * segsum_dst( (node_f @ W_side) * deg_out^-1/2 [src] )
Refactor used here (projection is linear, graph strictly bipartite):
    rst[d] = ( sum_{e->d} c_e * f_raw[src_e] ) @ W_side(d),
    c_e = deg_out[src]^-1/2 * deg_in[dst]^-1/2  (folded on host into scatter tiles)

Sharding: dst nodes dealt round-robin (degree-sorted) to 8 cores -> identical
compile-time schedule per core (SPMD), no collectives. Per core the device:
  1. dma_gather raw bf16 feature rows by src (256B rows). The gathers are
     spread across all 4 SWDGE queues: the Q7 ucode pins each queue's
     descriptor generation to one pair of GPSIMD cores (cpu_id/2 ==
     queue_num), so 4 in-flight gathers generate descriptors on 4 core
     pairs concurrently instead of serializing on pair 0.
  2. scatter-matmul (bf16): PSUM[128f, 512slot] += M_chunk[128e,128f].T
     @ S_chunk[128e,w] where S carries c_e at (edge_row, dst_col).
  3. projection matmul with the side weight (bf16 -> fp32 PSUM).
  4. feat-major fp32 output [128, slots]; host transposes/unpermutes.
"""
import sys
import os

for _p in ("/opt/trn_rl_repo",):
    if _p not in sys.path and os.path.isdir(_p):
        sys.path.insert(0, _p)

import numpy as np
import ml_dtypes

BF16 = ml_dtypes.bfloat16

N_U = 50000
N_V = 50000
N = N_U + N_V
D = 128
E = 1600000
N_CORES = 8
HALF = 25000          # int16-safe table window
WIN = 512             # dst slots per PSUM window
P = 128
NB = 4                # M/S bank depth (gather pipeline)
NQ = 4                # SWDGE queues (each owns one GPSIMD core pair)


# ----------------------------------------------------------------- host layout
def _build_layout(src, dst, cout, cin):
    """Canonical schedule + per-core edge/scatter data.

    Returns (schedule, per_core), where schedule is compile-time (identical
    across cores) and per_core holds idx/S arrays + output dst mapping.
    """
    layout_phases = []
    per_core_idx = [[] for _ in range(N_CORES)]
    per_core_sval = [[] for _ in range(N_CORES)]   # aligned with idx positions
    per_core_dsts = [[] for _ in range(N_CORES)]   # slot -> global dst id (-1 pad)

    for phase in range(2):
        if phase == 0:       # dsts are v-nodes, sources u-side
            mask = dst >= N_U
            d_local = dst[mask] - N_U
            s_local = src[mask]
            dst_base = N_U
        else:                # dsts are u-nodes, sources v-side
            mask = dst < N_U
            d_local = dst[mask]
            s_local = src[mask] - N_U
            dst_base = 0
        half = (s_local >= HALF).astype(np.int64)
        s_half_local = s_local - half * HALF

        n_dst = N_U
        a_cnt = np.bincount(d_local[half == 0], minlength=n_dst)
        b_cnt = np.bincount(d_local[half == 1], minlength=n_dst)

        order = np.lexsort((np.arange(n_dst), b_cnt, a_cnt))
        rank = np.empty(n_dst, np.int64)
        rank[order] = np.arange(n_dst)

        slots_per_core = (n_dst + N_CORES - 1) // N_CORES  # 6250
        # canonical per-slot degrees = max over cores (clipped >= 1)
        a_mat = np.zeros((N_CORES, slots_per_core), np.int64)
        b_mat = np.zeros((N_CORES, slots_per_core), np.int64)
        dst_mat = np.full((N_CORES, slots_per_core), -1, np.int64)
        r = np.arange(n_dst)
        a_mat[r % N_CORES, r // N_CORES] = a_cnt[order]
        b_mat[r % N_CORES, r // N_CORES] = b_cnt[order]
        dst_mat[r % N_CORES, r // N_CORES] = order + dst_base
        A = np.maximum(a_mat.max(axis=0), 1)
        B = np.maximum(b_mat.max(axis=0), 1)

        for k in range(N_CORES):
            per_core_dsts[k].append(dst_mat[k])

        # ---- canonical chunking per (window, pass), no slot straddles a chunk
        n_win = (slots_per_core + WIN - 1) // WIN
        windows = []
        # canonical edge-position base per slot, per pass
        pos_base = [np.zeros(slots_per_core, np.int64) for _ in (0, 1)]
        for w in range(n_win):
            s0, s1 = w * WIN, min((w + 1) * WIN, slots_per_core)
            wininfo = {"n_slots": s1 - s0, "passes": []}
            for p_i, C in enumerate((A, B)):
                chunks = []   # (col0, w, scol0)
                cur = 0       # fill in current chunk
                cur_chunk = None
                blocks = 0
                for s in range(s0, s1):
                    c = int(C[s])
                    if cur_chunk is None or cur + c > P:
                        if cur_chunk is not None:
                            chunks.append(cur_chunk)
                        cur_chunk = {"col0": s - s0, "cols": 0}
                        blocks += 1
                        cur = 0
                    pos_base[p_i][s] = (blocks - 1) * P + cur
                    cur += c
                    cur_chunk["cols"] = (s - s0) - cur_chunk["col0"] + 1
                if cur_chunk is not None:
                    chunks.append(cur_chunk)
                wininfo["passes"].append({"chunks": chunks, "n_blocks": blocks})
            windows.append(wininfo)
        layout_phases.append({
            "n_win": n_win,
            "slots_per_core": slots_per_core,
            "windows": windows,
        })

        # ---- per-core edge placement (vectorized)
        # rank within (dst, half) group:
        grp = d_local * 2 + half
        sort_i = np.argsort(grp, kind="stable")
        grp_s = grp[sort_i]
        starts = np.r_[0, np.nonzero(np.diff(grp_s))[0] + 1]
        group_id = np.cumsum(np.r_[0, (np.diff(grp_s) != 0).astype(np.int64)])
        first_pos_of_group = starts[group_id]
        within = np.arange(len(grp_s)) - first_pos_of_group
        e_rank = np.empty(len(grp), np.int64)
        e_rank[sort_i] = within

        e_core = rank[d_local] % N_CORES
        e_slot = rank[d_local] // N_CORES
        e_win = e_slot // WIN

        # global canonical position of each edge within its (win, pass) stream:
        e_pos = np.where(half == 0,
                         pos_base[0][e_slot],
                         pos_base[1][e_slot]) + e_rank

        # canonical call sizes (blocks) per (win, pass):
        call_blocks = np.array(
            [[windows[w]["passes"][p]["n_blocks"] for p in (0, 1)]
             for w in range(n_win)], np.int64)
        # canonical flat offsets: order = win-major, pass lo then hi
        call_sizes = (call_blocks * P).reshape(-1)           # [n_win*2]
        call_off = np.r_[0, np.cumsum(call_sizes)][:-1].reshape(n_win, 2)
        tot_idx = int(call_sizes.sum())

        # canonical S layout: per chunk scol0
        s_cols_per_call = []
        for w in range(n_win):
            for p_i in (0, 1):
                ch = windows[w]["passes"][p_i]["chunks"]
                cols = np.array([c["cols"] for c in ch], np.int64)
                s_cols_per_call.append(cols)
        chunk_cols_flat = np.concatenate(s_cols_per_call)
        chunk_scol0 = np.r_[0, np.cumsum(chunk_cols_flat)][:-1]
        tot_scols = int(chunk_cols_flat.sum())
        # record scol0 / col0 back into schedule for device build
        # (scol0 made global across phases via scol_phase_base)
        scol_phase_base = sum(
            pc.shape[1] for pc in per_core_sval[0]
        ) if per_core_sval[0] else 0
        ci = 0
        for w in range(n_win):
            for p_i in (0, 1):
                for c in windows[w]["passes"][p_i]["chunks"]:
                    c["scol0"] = int(chunk_scol0[ci]) + scol_phase_base
                    ci += 1

        # per-chunk col0 arrays for edge->scol math
        chunk_col0_flat = np.concatenate(
            [np.array([c["col0"] for c in windows[w]["passes"][p_i]["chunks"]],
                      np.int64)
             for w in range(n_win) for p_i in (0, 1)])
        # chunk global id for an edge: need per-call chunk base
        chunks_per_call = np.array([len(s) for s in s_cols_per_call], np.int64)
        call_chunk_base = np.r_[0, np.cumsum(chunks_per_call)][:-1].reshape(n_win, 2)

        e_call_off = call_off[e_win, half]
        e_gpos = e_call_off + e_pos                      # global idx position
        e_chunk = call_chunk_base[e_win, half] + e_pos // P
        e_row = e_pos % P
        e_scol = chunk_scol0[e_chunk] + (e_slot - e_win * WIN) - chunk_col0_flat[e_chunk]

        e_val = (cout[s_local + (0 if phase == 0 else N_U)]
                 * cin[d_local + dst_base]).astype(np.float32)

        for k in range(N_CORES):
            m = e_core == k
            idx_flat = np.zeros(tot_idx, np.int16)
            idx_flat[e_gpos[m]] = s_half_local[m].astype(np.int16)
            sv = np.zeros((P, tot_scols), BF16)
            sv[e_row[m], e_scol[m]] = e_val[m].astype(BF16)
            per_core_idx[k].append(idx_flat)
            per_core_sval[k].append(sv)

    # wrap idx per call into the [16, n/16].T-tiled layout, concat everything
    per_core = []
    for k in range(N_CORES):
        idx_cols = []
        for phase in range(2):
            ph = layout_phases[phase]
            flat = per_core_idx[k][phase]
            off = 0
            for w in range(ph["n_win"]):
                for p_i in (0, 1):
                    nb = ph["windows"][w]["passes"][p_i]["n_blocks"]
                    n = nb * P
                    call = flat[off:off + n]
                    off += n
                    t = call.reshape(n // 16, 16).T      # [16, n/16]
                    idx_cols.append(np.tile(t, (N_CORES, 1)))
        idx_arr = np.concatenate(idx_cols, axis=1)       # [128, tot/16]
        s_arr = np.concatenate(per_core_sval[k], axis=1)  # [128, scols]
        per_core.append({"idx": idx_arr, "s": s_arr, "dsts": per_core_dsts[k]})
    return layout_phases, per_core


# ------------------------------------------------------------------ device code
def _build_nc(sched):
    import concourse.bacc as bacc
    import concourse.bass as bass
    import concourse.mybir as mybir
    from concourse._compat import get_trn_type
    from concourse.library_config import mlp

    # 48KB/partition descriptor carveout: the default 16KB holds only 256
    # descs per SDMA engine ring, shared by all 4 SWDGE queues — a single
    # gather sub-call pushes ~160/engine, so generation stalls on ring space.
    nc = bacc.Bacc(get_trn_type() or "TRN2", target_bir_lowering=False,
                   debug=False, num_swdge_queues=NQ,
                   dynamic_dma_scratch_size=32768)
    f32 = mybir.dt.float32
    bf16 = mybir.dt.bfloat16
    u_f = nc.dram_tensor("u_f", [N_U, D], bf16, kind="ExternalInput")
    v_f = nc.dram_tensor("v_f", [N_V, D], bf16, kind="ExternalInput")
    u_w = nc.dram_tensor("u_w", [D, D], bf16, kind="ExternalInput")
    v_w = nc.dram_tensor("v_w", [D, D], bf16, kind="ExternalInput")

    # totals from schedule
    tot_idx_cols = 0
    tot_scols = 0
    tot_slots = 0
    nblk_max = 0
    for ph in sched:
        for w in ph["windows"]:
            tot_slots += w["n_slots"]
            nb = 0
            for p_i in (0, 1):
                pa = w["passes"][p_i]
                nb += pa["n_blocks"]
                tot_idx_cols += pa["n_blocks"] * P // 16
                tot_scols += sum(c["cols"] for c in pa["chunks"])
            nblk_max = max(nblk_max, nb)

    idx_in = nc.dram_tensor("idx", [P, tot_idx_cols], mybir.dt.int16,
                            kind="ExternalInput")
    s_in = nc.dram_tensor("sval", [P, tot_scols], bf16, kind="ExternalInput")
    out = nc.dram_tensor("out", [P, tot_slots], bf16, kind="ExternalOutput")

    idx_sb = nc.alloc_sbuf_tensor("idx_sb", [P, tot_idx_cols], mybir.dt.int16)
    m_sb = [nc.alloc_sbuf_tensor(f"m{i}", [P, nblk_max, P], bf16)
            for i in range(NB)]
    s_sb = [nc.alloc_sbuf_tensor(f"s{i}", [P, 2 * WIN], bf16)
            for i in range(NB)]
    agg_sb = [nc.alloc_sbuf_tensor(f"agg{i}", [P, WIN], bf16) for i in (0, 1)]
    stage_sb = nc.alloc_sbuf_tensor("stage", [P, tot_slots], bf16)
    w_sb = [nc.alloc_sbuf_tensor(f"w{i}", [P, D], bf16) for i in (0, 1)]

    agg_ps = [nc.alloc_psum_tensor(f"aps{i}", [P, WIN], f32) for i in (0, 1)]
    proj_ps = [nc.alloc_psum_tensor(f"pps{i}", [P, WIN], f32) for i in (0, 1)]

    sem_ld = nc.alloc_semaphore("ld")        # upfront loads + final store
    sem_idx = nc.alloc_semaphore("idxld")    # idx table load
    sem_s = [nc.alloc_semaphore(f"ssem{i}") for i in range(NB)]
    sem_g = [nc.alloc_semaphore(f"gsem{i}") for i in range(NB)]
    sem_mm = [nc.alloc_semaphore(f"mmsem{i}") for i in range(NB)]
    sem_agg = [nc.alloc_semaphore(f"aggsem{i}") for i in (0, 1)]
    sem_proj = [nc.alloc_semaphore(f"projsem{i}") for i in (0, 1)]
    sem_stage = [nc.alloc_semaphore(f"stsem{i}") for i in (0, 1)]

    # flatten windows across phases into one global list
    wlist = []
    icol = 0
    scol = 0
    slot0 = 0
    for phase, ph in enumerate(sched):
        for w in ph["windows"]:
            entry = {"phase": phase, "n_slots": w["n_slots"], "passes": [],
                     "slot0": slot0}
            for p_i in (0, 1):
                pa = w["passes"][p_i]
                nb = pa["n_blocks"]
                entry["passes"].append({
                    "icol": icol, "nb": nb,
                    "chunks": pa["chunks"], "scol": scol,
                })
                icol += nb * P // 16
                scol += sum(c["cols"] for c in pa["chunks"])
            slot0 += w["n_slots"]
            wlist.append(entry)
    NW = len(wlist)
    n_even = (NW + 1) // 2
    n_odd = NW // 2

    # Queue-cost-weighted gather sub-calls: the Q7 gather ucode's idx read
    # streams (queue_num+1)*32 SBUF channels, so per-block cost rises with
    # queue number. Give low queues proportionally more blocks so all 4
    # core pairs finish together (in-order completion locksteps each
    # window's quad on its slowest member).
    QW = (0.40, 0.26, 0.19, 0.15)

    for went in wlist:
        nb_lo = went["passes"][0]["nb"]
        nb_hi = went["passes"][1]["nb"]
        tot = nb_lo + nb_hi
        # largest-remainder rounding of tot * QW into integer block quotas
        raw = [tot * w for w in QW]
        quo = [int(x) for x in raw]
        rem = tot - sum(quo)
        for i in sorted(range(NQ), key=lambda i: quo[i] - raw[i])[:rem]:
            quo[i] += 1
        # deal quotas out of the [lo | hi] block sequence; a quota that
        # straddles the lo/hi table boundary becomes two calls on one queue
        subs = []   # (queue, p_i, blk_s, blk_e)
        pos = 0
        for q in range(NQ):
            take = quo[q]
            while take > 0:
                if pos < nb_lo:
                    n = min(take, nb_lo - pos)
                    subs.append((q, 0, pos, pos + n))
                else:
                    n = min(take, tot - pos)
                    subs.append((q, 1, pos - nb_lo, pos - nb_lo + n))
                pos += n
                take -= n
        went["subs"] = subs

    # idx preload split point (after window 3) for a faster pipeline start
    idx_c0 = wlist[4]["passes"][0]["icol"] if NW > 4 else tot_idx_cols
    # output stream pieces: after these windows, flush finished stage slots
    flush_after = sorted(set([NW // 4 - 1, NW // 2 - 1, 3 * NW // 4 - 1, NW - 1]))
    flush_after = [f for f in flush_after if f >= 0]

    with nc.Block() as block:
        @block.sync
        def _(sy: bass.BassEngine):
            sy.dma_start(idx_sb[:, :idx_c0], idx_in[:, :idx_c0]).then_inc(
                sem_idx, 16)
            sy.dma_start(w_sb[0][:], u_w[:]).then_inc(sem_ld, 16)
            sy.dma_start(w_sb[1][:], v_w[:]).then_inc(sem_ld, 16)
            if idx_c0 < tot_idx_cols:
                sy.dma_start(idx_sb[:, idx_c0:], idx_in[:, idx_c0:]).then_inc(
                    sem_idx, 16)
            n_flush = 0
            flush_slot0 = 0
            for wi, went in enumerate(wlist):
                b = wi % NB
                # WAR: S bank b free after window wi-NB's matmuls done
                if wi >= NB:
                    sy.wait_ge(sem_mm[b], wi // NB)
                p0, p1 = went["passes"]
                ncols = (sum(c["cols"] for c in p0["chunks"])
                         + sum(c["cols"] for c in p1["chunks"]))
                sy.dma_start(
                    s_sb[b][:, :ncols], s_in[:, p0["scol"]:p0["scol"] + ncols]
                ).then_inc(sem_s[b], 16)
                if wi in flush_after:
                    # stream finished stage slots out while later windows run
                    sy.wait_ge(sem_stage[0], wi // 2 + 1)
                    sy.wait_ge(sem_stage[1], (wi + 1) // 2)
                    end = went["slot0"] + went["n_slots"]
                    sy.dma_start(
                        out[:, flush_slot0:end], stage_sb[:, flush_slot0:end]
                    ).then_inc(sem_ld, 16)
                    flush_slot0 = end
                    n_flush += 1
            sy.wait_ge(sem_ld, 32 + 16 * n_flush)

        @block.gpsimd
        def _(gp: bass.BassGpSimd):
            gp.load_library(mlp)
            for wi, went in enumerate(wlist):
                b = wi % NB
                phase = went["phase"]
                if phase == 0:
                    tab_lo, tab_hi = u_f[0:HALF, :], u_f[HALF:N_U, :]
                else:
                    tab_lo, tab_hi = v_f[0:HALF, :], v_f[HALF:N_V, :]
                nb_lo = went["passes"][0]["nb"]
                # fold pipeline waits into the first gather of the window: a
                # standalone wait occupies a slot in the 4-deep in-order Pool
                # completion window and caps the number of concurrently-
                # generating Q7 core pairs at 3. One wait slot per
                # instruction; any extras go standalone (rare).
                waits = []
                if wi == 0:
                    waits.append((sem_idx, 16))
                if wi == 4 and idx_c0 < tot_idx_cols:
                    waits.append((sem_idx, 32))
                if wi >= NB:
                    waits.append((sem_mm[b], wi // NB))
                for sem, val in waits[1:]:
                    gp.wait_ge(sem, val)
                for si, (q, p_i, blk_s, blk_e) in enumerate(went["subs"]):
                    pa = went["passes"][p_i]
                    tab = tab_lo if p_i == 0 else tab_hi
                    nsub = (blk_e - blk_s) * P
                    blk_base = (0 if p_i == 0 else nb_lo) + blk_s
                    icol_s = pa["icol"] + blk_s * P // 16
                    inst = gp.dma_gather(
                        m_sb[b][:, blk_base:blk_base + (blk_e - blk_s), :],
                        tab,
                        idx_sb[:, icol_s:icol_s + nsub // 16],
                        nsub, nsub, D,
                        single_packet=False,
                        queue_num=q,
                    ).then_inc(sem_g[b], 16)
                    if si == 0 and waits:
                        inst.wait_op(*waits[0], "sem-ge")

        @block.tensor
        def _(te):
            g_seen = [0] * NB
            s_seen = [0] * NB
            for wi, went in enumerate(wlist):
                b = wi % NB
                pb = wi % 2
                phase = went["phase"]
                # all gather sub-calls + S stream for this window
                g_seen[b] += 16 * len(went["subs"])
                s_seen[b] += 16
                te.wait_ge(sem_g[b], g_seen[b])
                te.wait_ge(sem_s[b], s_seen[b])
                # agg_ps WAR vs vector copy of window wi-2
                if wi >= 2:
                    te.wait_ge(sem_agg[pb], wi // 2)
                ns = went["n_slots"]
                blk0 = 0
                last = None
                for p_i in (0, 1):
                    pa = went["passes"][p_i]
                    for ci, ch in enumerate(pa["chunks"]):
                        last = (p_i, ci)
                first = True
                for p_i in (0, 1):
                    pa = went["passes"][p_i]
                    for ci, ch in enumerate(pa["chunks"]):
                        blk = blk0 + ci
                        sc = ch["scol0"] - went["passes"][0]["scol"]
                        mm = te.matmul(
                            out=agg_ps[pb][:, ch["col0"]:ch["col0"] + ch["cols"]],
                            lhsT=m_sb[b][:, blk, :],
                            rhs=s_sb[b][:, sc:sc + ch["cols"]],
                            start=first,
                            stop=((p_i, ci) == last),
                        )
                        first = False
                        if (p_i, ci) == last:
                            mm.then_inc(sem_mm[b], 1)
                    blk0 += pa["nb"]
                # projection: wait for vector to copy agg->sbuf (this window)
                te.wait_ge(sem_agg[pb], wi // 2 + 1)
                # proj_ps WAR vs vector stage copy of window wi-2
                if wi >= 2:
                    te.wait_ge(sem_stage[pb], wi // 2)
                te.matmul(
                    out=proj_ps[pb][:, :ns],
                    lhsT=w_sb[phase][:],
                    rhs=agg_sb[pb][:, :ns],
                    start=True, stop=True,
                ).then_inc(sem_proj[pb], 1)

        @block.vector
        def _(ve):
            for wi, went in enumerate(wlist):
                b = wi % NB
                pb = wi % 2
                ns = went["n_slots"]
                ve.wait_ge(sem_mm[b], wi // NB + 1)
                ve.tensor_copy(out=agg_sb[pb][:, :ns],
                               in_=agg_ps[pb][:, :ns]).then_inc(sem_agg[pb], 1)
                ve.wait_ge(sem_proj[pb], wi // 2 + 1)
                ve.tensor_copy(
                    out=stage_sb[:, went["slot0"]:went["slot0"] + ns],
                    in_=proj_ps[pb][:, :ns],
                ).then_inc(sem_stage[pb], 1)

    nc.compile()
    return nc


# ---------------------------------------------------------------------- kernel
def kernel(u_f, v_f, u_w, v_w, src, dst):
    from concourse.bass_utils import run_bass_kernel_spmd

    src = np.asarray(src)
    dst = np.asarray(dst)
    u_f = np.asarray(u_f, np.float32)
    v_f = np.asarray(v_f, np.float32)

    deg_out = np.bincount(src, minlength=N).astype(np.float32)
    deg_in = np.bincount(dst, minlength=N).astype(np.float32)
    cout = np.maximum(deg_out, 1.0) ** -0.5
    cin = np.maximum(deg_in, 1.0) ** -0.5

    sched, per_core = _build_layout(src, dst, cout, cin)

    nc = _build_nc(sched)
    u_f_b = u_f.astype(BF16)
    v_f_b = v_f.astype(BF16)
    u_w_b = np.asarray(u_w, np.float32).astype(BF16)
    v_w_b = np.asarray(v_w, np.float32).astype(BF16)
    in_maps = []
    for k in range(N_CORES):
        in_maps.append({
            "u_f": u_f_b, "v_f": v_f_b,
            "u_w": u_w_b, "v_w": v_w_b,
            "idx": per_core[k]["idx"], "sval": per_core[k]["s"],
        })
    trace = bool(os.environ.get("KERNEL_TRACE"))
    res = run_bass_kernel_spmd(nc, in_maps, core_ids=list(range(N_CORES)),
                               trace=trace)
    if trace:
        print(f"HW exec time: {res.exec_time_ns} ns")
        kernel.last_profile = res.profile_json

    out_full = np.zeros((N, D), np.float32)
    for k in range(N_CORES):
        fm = np.asarray(res.results[k]["out"], dtype=np.float32)
        rows = np.ascontiguousarray(fm.T)     # [tot_slots, 128]
        slot0 = 0
        for phase in range(2):
            dsts = per_core[k]["dsts"][phase]
            nslots = len(dsts)
            valid = dsts >= 0
            out_full[dsts[valid]] = rows[slot0:slot0 + nslots][valid]
            slot0 += nslots
    return out_full


# revision 13
# speedup vs baseline: 1.6839x; 1.6839x over previous
"""Bipartite GCN message-passing kernel for 8 Trainium2 NeuronCores.

Math (reference): rst = deg_in^-1/2 * segsum_dst( (node_f @ W_side) * deg_out^-1/2 [src] )
Refactor used here (projection is linear, graph strictly bipartite):
    rst[d] = ( sum_{e->d} c_e * f_raw[src_e] ) @ W_side(d),
    c_e = deg_out[src]^-1/2 * deg_in[dst]^-1/2  (folded on host into scatter tiles)

Sharding: dst nodes dealt round-robin (degree-sorted) to 8 cores -> identical
compile-time schedule per core (SPMD), no collectives. Per core the device:
  1. dma_gather raw bf16 feature rows by src (256B rows). The gathers are
     spread across all 4 SWDGE queues: the Q7 ucode pins each queue's
     descriptor generation to one pair of GPSIMD cores (cpu_id/2 ==
     queue_num), so 4 in-flight gathers generate descriptors on 4 core
     pairs concurrently instead of serializing on pair 0.
  2. scatter-matmul (bf16): PSUM[128f, 512slot] += M_chunk[128e,128f].T
     @ S_chunk[128e,w] where S carries c_e at (edge_row, dst_col).
  3. projection matmul with the side weight (bf16 -> fp32 PSUM).
  4. feat-major fp32 output [128, slots]; host transposes/unpermutes.
"""
import sys
import os

for _p in ("/opt/trn_rl_repo",):
    if _p not in sys.path and os.path.isdir(_p):
        sys.path.insert(0, _p)

import numpy as np
import ml_dtypes

BF16 = ml_dtypes.bfloat16

N_U = 50000
N_V = 50000
N = N_U + N_V
D = 128
E = 1600000
N_CORES = 8
HALF = 25000          # int16-safe table window
WIN = 512             # dst slots per PSUM window
P = 128
NB = 4                # M/S bank depth (gather pipeline)
NQ = 4                # SWDGE queues (each owns one GPSIMD core pair)


# ----------------------------------------------------------------- host layout
def _build_layout(src, dst, cout, cin):
    """Canonical schedule + per-core edge/scatter data.

    Returns (schedule, per_core), where schedule is compile-time (identical
    across cores) and per_core holds idx/S arrays + output dst mapping.
    """
    layout_phases = []
    per_core_idx = [[] for _ in range(N_CORES)]
    per_core_sval = [[] for _ in range(N_CORES)]   # aligned with idx positions
    per_core_dsts = [[] for _ in range(N_CORES)]   # slot -> global dst id (-1 pad)

    for phase in range(2):
        if phase == 0:       # dsts are v-nodes, sources u-side
            mask = dst >= N_U
            d_local = dst[mask] - N_U
            s_local = src[mask]
            dst_base = N_U
        else:                # dsts are u-nodes, sources v-side
            mask = dst < N_U
            d_local = dst[mask]
            s_local = src[mask] - N_U
            dst_base = 0
        half = (s_local >= HALF).astype(np.int64)
        s_half_local = s_local - half * HALF

        n_dst = N_U
        a_cnt = np.bincount(d_local[half == 0], minlength=n_dst)
        b_cnt = np.bincount(d_local[half == 1], minlength=n_dst)

        order = np.lexsort((np.arange(n_dst), b_cnt, a_cnt))
        rank = np.empty(n_dst, np.int64)
        rank[order] = np.arange(n_dst)

        slots_per_core = (n_dst + N_CORES - 1) // N_CORES  # 6250
        # canonical per-slot degrees = max over cores (clipped >= 1)
        a_mat = np.zeros((N_CORES, slots_per_core), np.int64)
        b_mat = np.zeros((N_CORES, slots_per_core), np.int64)
        dst_mat = np.full((N_CORES, slots_per_core), -1, np.int64)
        r = np.arange(n_dst)
        a_mat[r % N_CORES, r // N_CORES] = a_cnt[order]
        b_mat[r % N_CORES, r // N_CORES] = b_cnt[order]
        dst_mat[r % N_CORES, r // N_CORES] = order + dst_base
        A = np.maximum(a_mat.max(axis=0), 1)
        B = np.maximum(b_mat.max(axis=0), 1)

        for k in range(N_CORES):
            per_core_dsts[k].append(dst_mat[k])

        # ---- canonical chunking per (window, pass), no slot straddles a chunk
        n_win = (slots_per_core + WIN - 1) // WIN
        windows = []
        # canonical edge-position base per slot, per pass
        pos_base = [np.zeros(slots_per_core, np.int64) for _ in (0, 1)]
        for w in range(n_win):
            s0, s1 = w * WIN, min((w + 1) * WIN, slots_per_core)
            wininfo = {"n_slots": s1 - s0, "passes": []}
            for p_i, C in enumerate((A, B)):
                chunks = []   # (col0, w, scol0)
                cur = 0       # fill in current chunk
                cur_chunk = None
                blocks = 0
                for s in range(s0, s1):
                    c = int(C[s])
                    if cur_chunk is None or cur + c > P:
                        if cur_chunk is not None:
                            chunks.append(cur_chunk)
                        cur_chunk = {"col0": s - s0, "cols": 0}
                        blocks += 1
                        cur = 0
                    pos_base[p_i][s] = (blocks - 1) * P + cur
                    cur += c
                    cur_chunk["cols"] = (s - s0) - cur_chunk["col0"] + 1
                if cur_chunk is not None:
                    chunks.append(cur_chunk)
                wininfo["passes"].append({"chunks": chunks, "n_blocks": blocks})
            windows.append(wininfo)
        layout_phases.append({
            "n_win": n_win,
            "slots_per_core": slots_per_core,
            "windows": windows,
        })

        # ---- per-core edge placement (vectorized)
        # rank within (dst, half) group:
        grp = d_local * 2 + half
        sort_i = np.argsort(grp, kind="stable")
        grp_s = grp[sort_i]
        starts = np.r_[0, np.nonzero(np.diff(grp_s))[0] + 1]
        group_id = np.cumsum(np.r_[0, (np.diff(grp_s) != 0).astype(np.int64)])
        first_pos_of_group = starts[group_id]
        within = np.arange(len(grp_s)) - first_pos_of_group
        e_rank = np.empty(len(grp), np.int64)
        e_rank[sort_i] = within

        e_core = rank[d_local] % N_CORES
        e_slot = rank[d_local] // N_CORES
        e_win = e_slot // WIN

        # global canonical position of each edge within its (win, pass) stream:
        e_pos = np.where(half == 0,
                         pos_base[0][e_slot],
                         pos_base[1][e_slot]) + e_rank

        # canonical call sizes (blocks) per (win, pass):
        call_blocks = np.array(
            [[windows[w]["passes"][p]["n_blocks"] for p in (0, 1)]
             for w in range(n_win)], np.int64)
        # canonical flat offsets: order = win-major, pass lo then hi
        call_sizes = (call_blocks * P).reshape(-1)           # [n_win*2]
        call_off = np.r_[0, np.cumsum(call_sizes)][:-1].reshape(n_win, 2)
        tot_idx = int(call_sizes.sum())

        # canonical S layout: per chunk scol0
        s_cols_per_call = []
        for w in range(n_win):
            for p_i in (0, 1):
                ch = windows[w]["passes"][p_i]["chunks"]
                cols = np.array([c["cols"] for c in ch], np.int64)
                s_cols_per_call.append(cols)
        chunk_cols_flat = np.concatenate(s_cols_per_call)
        chunk_scol0 = np.r_[0, np.cumsum(chunk_cols_flat)][:-1]
        tot_scols = int(chunk_cols_flat.sum())
        # record scol0 / col0 back into schedule for device build
        # (scol0 made global across phases via scol_phase_base)
        scol_phase_base = sum(
            pc.shape[1] for pc in per_core_sval[0]
        ) if per_core_sval[0] else 0
        ci = 0
        for w in range(n_win):
            for p_i in (0, 1):
                for c in windows[w]["passes"][p_i]["chunks"]:
                    c["scol0"] = int(chunk_scol0[ci]) + scol_phase_base
                    ci += 1

        # per-chunk col0 arrays for edge->scol math
        chunk_col0_flat = np.concatenate(
            [np.array([c["col0"] for c in windows[w]["passes"][p_i]["chunks"]],
                      np.int64)
             for w in range(n_win) for p_i in (0, 1)])
        # chunk global id for an edge: need per-call chunk base
        chunks_per_call = np.array([len(s) for s in s_cols_per_call], np.int64)
        call_chunk_base = np.r_[0, np.cumsum(chunks_per_call)][:-1].reshape(n_win, 2)

        e_call_off = call_off[e_win, half]
        e_gpos = e_call_off + e_pos                      # global idx position
        e_chunk = call_chunk_base[e_win, half] + e_pos // P
        e_row = e_pos % P
        e_scol = chunk_scol0[e_chunk] + (e_slot - e_win * WIN) - chunk_col0_flat[e_chunk]

        e_val = (cout[s_local + (0 if phase == 0 else N_U)]
                 * cin[d_local + dst_base]).astype(np.float32)

        for k in range(N_CORES):
            m = e_core == k
            idx_flat = np.zeros(tot_idx, np.int16)
            idx_flat[e_gpos[m]] = s_half_local[m].astype(np.int16)
            sv = np.zeros((P, tot_scols), BF16)
            sv[e_row[m], e_scol[m]] = e_val[m].astype(BF16)
            per_core_idx[k].append(idx_flat)
            per_core_sval[k].append(sv)

    # wrap idx per call into the [16, n/16].T-tiled layout, concat everything
    per_core = []
    for k in range(N_CORES):
        idx_cols = []
        for phase in range(2):
            ph = layout_phases[phase]
            flat = per_core_idx[k][phase]
            off = 0
            for w in range(ph["n_win"]):
                for p_i in (0, 1):
                    nb = ph["windows"][w]["passes"][p_i]["n_blocks"]
                    n = nb * P
                    call = flat[off:off + n]
                    off += n
                    t = call.reshape(n // 16, 16).T      # [16, n/16]
                    idx_cols.append(np.tile(t, (N_CORES, 1)))
        idx_arr = np.concatenate(idx_cols, axis=1)       # [128, tot/16]
        s_arr = np.concatenate(per_core_sval[k], axis=1)  # [128, scols]
        per_core.append({"idx": idx_arr, "s": s_arr, "dsts": per_core_dsts[k]})
    return layout_phases, per_core


# ------------------------------------------------------------------ device code
def _build_nc(sched):
    import concourse.bacc as bacc
    import concourse.bass as bass
    import concourse.mybir as mybir
    from concourse._compat import get_trn_type
    from concourse.library_config import mlp

    nc = bacc.Bacc(get_trn_type() or "TRN2", target_bir_lowering=False,
                   debug=False, num_swdge_queues=NQ)
    f32 = mybir.dt.float32
    bf16 = mybir.dt.bfloat16
    u_f = nc.dram_tensor("u_f", [N_U, D], bf16, kind="ExternalInput")
    v_f = nc.dram_tensor("v_f", [N_V, D], bf16, kind="ExternalInput")
    u_w = nc.dram_tensor("u_w", [D, D], bf16, kind="ExternalInput")
    v_w = nc.dram_tensor("v_w", [D, D], bf16, kind="ExternalInput")

    # totals from schedule
    tot_idx_cols = 0
    tot_scols = 0
    tot_slots = 0
    nblk_max = 0
    for ph in sched:
        for w in ph["windows"]:
            tot_slots += w["n_slots"]
            nb = 0
            for p_i in (0, 1):
                pa = w["passes"][p_i]
                nb += pa["n_blocks"]
                tot_idx_cols += pa["n_blocks"] * P // 16
                tot_scols += sum(c["cols"] for c in pa["chunks"])
            nblk_max = max(nblk_max, nb)

    idx_in = nc.dram_tensor("idx", [P, tot_idx_cols], mybir.dt.int16,
                            kind="ExternalInput")
    s_in = nc.dram_tensor("sval", [P, tot_scols], bf16, kind="ExternalInput")
    out = nc.dram_tensor("out", [P, tot_slots], bf16, kind="ExternalOutput")

    idx_sb = nc.alloc_sbuf_tensor("idx_sb", [P, tot_idx_cols], mybir.dt.int16)
    m_sb = [nc.alloc_sbuf_tensor(f"m{i}", [P, nblk_max, P], bf16)
            for i in range(NB)]
    s_sb = [nc.alloc_sbuf_tensor(f"s{i}", [P, 2 * WIN], bf16)
            for i in range(NB)]
    agg_sb = [nc.alloc_sbuf_tensor(f"agg{i}", [P, WIN], bf16) for i in (0, 1)]
    stage_sb = nc.alloc_sbuf_tensor("stage", [P, tot_slots], bf16)
    w_sb = [nc.alloc_sbuf_tensor(f"w{i}", [P, D], bf16) for i in (0, 1)]

    agg_ps = [nc.alloc_psum_tensor(f"aps{i}", [P, WIN], f32) for i in (0, 1)]
    proj_ps = [nc.alloc_psum_tensor(f"pps{i}", [P, WIN], f32) for i in (0, 1)]

    sem_ld = nc.alloc_semaphore("ld")        # upfront loads + final store
    sem_idx = nc.alloc_semaphore("idxld")    # idx table load
    sem_s = [nc.alloc_semaphore(f"ssem{i}") for i in range(NB)]
    sem_g = [nc.alloc_semaphore(f"gsem{i}") for i in range(NB)]
    sem_mm = [nc.alloc_semaphore(f"mmsem{i}") for i in range(NB)]
    sem_agg = [nc.alloc_semaphore(f"aggsem{i}") for i in (0, 1)]
    sem_proj = [nc.alloc_semaphore(f"projsem{i}") for i in (0, 1)]
    sem_stage = [nc.alloc_semaphore(f"stsem{i}") for i in (0, 1)]

    # flatten windows across phases into one global list
    wlist = []
    icol = 0
    scol = 0
    slot0 = 0
    for phase, ph in enumerate(sched):
        for w in ph["windows"]:
            entry = {"phase": phase, "n_slots": w["n_slots"], "passes": [],
                     "slot0": slot0}
            for p_i in (0, 1):
                pa = w["passes"][p_i]
                nb = pa["n_blocks"]
                entry["passes"].append({
                    "icol": icol, "nb": nb,
                    "chunks": pa["chunks"], "scol": scol,
                })
                icol += nb * P // 16
                scol += sum(c["cols"] for c in pa["chunks"])
            slot0 += w["n_slots"]
            wlist.append(entry)
    NW = len(wlist)
    n_even = (NW + 1) // 2
    n_odd = NW // 2

    # Balanced gather sub-calls: 4 per window (one per SWDGE queue), sizes
    # as equal as possible so the 4 Q7 core pairs finish together (in-order
    # completion with ~4 outstanding Pool instructions locksteps each
    # quad on its slowest member).
    def _split(n, k):
        q, r = divmod(n, k)
        return [q + (i < r) for i in range(k)]

    for wi, went in enumerate(wlist):
        nb_lo = went["passes"][0]["nb"]
        nb_hi = went["passes"][1]["nb"]
        tot = nb_lo + nb_hi
        if nb_lo == 0:
            a = 0
        elif nb_hi == 0:
            a = 4
        else:
            a = min(3, max(1, round(4 * nb_lo / tot)))
        subs = []   # (queue, p_i, blk_s, blk_e)
        qi = 0
        for p_i, npart, nb in ((0, a, nb_lo), (1, 4 - a, nb_hi)):
            if npart == 0 or nb == 0:
                continue
            s = 0
            for sz in _split(nb, npart):
                if sz:
                    subs.append((qi % NQ, p_i, s, s + sz))
                    qi += 1
                    s += sz
        went["subs"] = subs

    # idx preload split point (after window 3) for a faster pipeline start
    idx_c0 = wlist[4]["passes"][0]["icol"] if NW > 4 else tot_idx_cols
    # output stream pieces: after these windows, flush finished stage slots
    flush_after = sorted(set([NW // 4 - 1, NW // 2 - 1, 3 * NW // 4 - 1, NW - 1]))
    flush_after = [f for f in flush_after if f >= 0]

    with nc.Block() as block:
        @block.sync
        def _(sy: bass.BassEngine):
            sy.dma_start(idx_sb[:, :idx_c0], idx_in[:, :idx_c0]).then_inc(
                sem_idx, 16)
            sy.dma_start(w_sb[0][:], u_w[:]).then_inc(sem_ld, 16)
            sy.dma_start(w_sb[1][:], v_w[:]).then_inc(sem_ld, 16)
            if idx_c0 < tot_idx_cols:
                sy.dma_start(idx_sb[:, idx_c0:], idx_in[:, idx_c0:]).then_inc(
                    sem_idx, 16)
            n_flush = 0
            flush_slot0 = 0
            for wi, went in enumerate(wlist):
                b = wi % NB
                # WAR: S bank b free after window wi-NB's matmuls done
                if wi >= NB:
                    sy.wait_ge(sem_mm[b], wi // NB)
                p0, p1 = went["passes"]
                ncols = (sum(c["cols"] for c in p0["chunks"])
                         + sum(c["cols"] for c in p1["chunks"]))
                sy.dma_start(
                    s_sb[b][:, :ncols], s_in[:, p0["scol"]:p0["scol"] + ncols]
                ).then_inc(sem_s[b], 16)
                if wi in flush_after:
                    # stream finished stage slots out while later windows run
                    sy.wait_ge(sem_stage[0], wi // 2 + 1)
                    sy.wait_ge(sem_stage[1], (wi + 1) // 2)
                    end = went["slot0"] + went["n_slots"]
                    sy.dma_start(
                        out[:, flush_slot0:end], stage_sb[:, flush_slot0:end]
                    ).then_inc(sem_ld, 16)
                    flush_slot0 = end
                    n_flush += 1
            sy.wait_ge(sem_ld, 32 + 16 * n_flush)

        @block.gpsimd
        def _(gp: bass.BassGpSimd):
            gp.load_library(mlp)
            for wi, went in enumerate(wlist):
                b = wi % NB
                phase = went["phase"]
                if phase == 0:
                    tab_lo, tab_hi = u_f[0:HALF, :], u_f[HALF:N_U, :]
                else:
                    tab_lo, tab_hi = v_f[0:HALF, :], v_f[HALF:N_V, :]
                nb_lo = went["passes"][0]["nb"]
                if wi == 0:
                    gp.wait_ge(sem_idx, 16)
                if wi == 4 and idx_c0 < tot_idx_cols:
                    gp.wait_ge(sem_idx, 32)
                if wi >= NB:
                    gp.wait_ge(sem_mm[b], wi // NB)
                for q, p_i, blk_s, blk_e in went["subs"]:
                    pa = went["passes"][p_i]
                    tab = tab_lo if p_i == 0 else tab_hi
                    nsub = (blk_e - blk_s) * P
                    blk_base = (0 if p_i == 0 else nb_lo) + blk_s
                    icol_s = pa["icol"] + blk_s * P // 16
                    gp.dma_gather(
                        m_sb[b][:, blk_base:blk_base + (blk_e - blk_s), :],
                        tab,
                        idx_sb[:, icol_s:icol_s + nsub // 16],
                        nsub, nsub, D,
                        single_packet=False,
                        queue_num=q,
                    ).then_inc(sem_g[b], 16)

        @block.tensor
        def _(te):
            g_seen = [0] * NB
            s_seen = [0] * NB
            for wi, went in enumerate(wlist):
                b = wi % NB
                pb = wi % 2
                phase = went["phase"]
                # all gather sub-calls + S stream for this window
                g_seen[b] += 16 * len(went["subs"])
                s_seen[b] += 16
                te.wait_ge(sem_g[b], g_seen[b])
                te.wait_ge(sem_s[b], s_seen[b])
                # agg_ps WAR vs vector copy of window wi-2
                if wi >= 2:
                    te.wait_ge(sem_agg[pb], wi // 2)
                ns = went["n_slots"]
                blk0 = 0
                last = None
                for p_i in (0, 1):
                    pa = went["passes"][p_i]
                    for ci, ch in enumerate(pa["chunks"]):
                        last = (p_i, ci)
                first = True
                for p_i in (0, 1):
                    pa = went["passes"][p_i]
                    for ci, ch in enumerate(pa["chunks"]):
                        blk = blk0 + ci
                        sc = ch["scol0"] - went["passes"][0]["scol"]
                        mm = te.matmul(
                            out=agg_ps[pb][:, ch["col0"]:ch["col0"] + ch["cols"]],
                            lhsT=m_sb[b][:, blk, :],
                            rhs=s_sb[b][:, sc:sc + ch["cols"]],
                            start=first,
                            stop=((p_i, ci) == last),
                        )
                        first = False
                        if (p_i, ci) == last:
                            mm.then_inc(sem_mm[b], 1)
                    blk0 += pa["nb"]
                # projection: wait for vector to copy agg->sbuf (this window)
                te.wait_ge(sem_agg[pb], wi // 2 + 1)
                # proj_ps WAR vs vector stage copy of window wi-2
                if wi >= 2:
                    te.wait_ge(sem_stage[pb], wi // 2)
                te.matmul(
                    out=proj_ps[pb][:, :ns],
                    lhsT=w_sb[phase][:],
                    rhs=agg_sb[pb][:, :ns],
                    start=True, stop=True,
                ).then_inc(sem_proj[pb], 1)

        @block.vector
        def _(ve):
            for wi, went in enumerate(wlist):
                b = wi % NB
                pb = wi % 2
                ns = went["n_slots"]
                ve.wait_ge(sem_mm[b], wi // NB + 1)
                ve.tensor_copy(out=agg_sb[pb][:, :ns],
                               in_=agg_ps[pb][:, :ns]).then_inc(sem_agg[pb], 1)
                ve.wait_ge(sem_proj[pb], wi // 2 + 1)
                ve.tensor_copy(
                    out=stage_sb[:, went["slot0"]:went["slot0"] + ns],
                    in_=proj_ps[pb][:, :ns],
                ).then_inc(sem_stage[pb], 1)

    nc.compile()
    return nc


# ---------------------------------------------------------------------- kernel
def kernel(u_f, v_f, u_w, v_w, src, dst):
    from concourse.bass_utils import run_bass_kernel_spmd

    src = np.asarray(src)
    dst = np.asarray(dst)
    u_f = np.asarray(u_f, np.float32)
    v_f = np.asarray(v_f, np.float32)

    deg_out = np.bincount(src, minlength=N).astype(np.float32)
    deg_in = np.bincount(dst, minlength=N).astype(np.float32)
    cout = np.maximum(deg_out, 1.0) ** -0.5
    cin = np.maximum(deg_in, 1.0) ** -0.5

    sched, per_core = _build_layout(src, dst, cout, cin)

    nc = _build_nc(sched)
    u_f_b = u_f.astype(BF16)
    v_f_b = v_f.astype(BF16)
    u_w_b = np.asarray(u_w, np.float32).astype(BF16)
    v_w_b = np.asarray(v_w, np.float32).astype(BF16)
    in_maps = []
    for k in range(N_CORES):
        in_maps.append({
            "u_f": u_f_b, "v_f": v_f_b,
            "u_w": u_w_b, "v_w": v_w_b,
            "idx": per_core[k]["idx"], "sval": per_core[k]["s"],
        })
    trace = bool(os.environ.get("KERNEL_TRACE"))
    res = run_bass_kernel_spmd(nc, in_maps, core_ids=list(range(N_CORES)),
                               trace=trace)
    if trace:
        print(f"HW exec time: {res.exec_time_ns} ns")
        kernel.last_profile = res.profile_json

    out_full = np.zeros((N, D), np.float32)
    for k in range(N_CORES):
        fm = np.asarray(res.results[k]["out"], dtype=np.float32)
        rows = np.ascontiguousarray(fm.T)     # [tot_slots, 128]
        slot0 = 0
        for phase in range(2):
            dsts = per_core[k]["dsts"][phase]
            nslots = len(dsts)
            valid = dsts >= 0
            out_full[dsts[valid]] = rows[slot0:slot0 + nslots][valid]
            slot0 += nslots
    return out_full


# revision 14
# speedup vs baseline: 1.6986x; 1.0087x over previous
"""Bipartite GCN message-passing kernel for 8 Trainium2 NeuronCores.

Math (reference): rst = deg_in^-1/2 * segsum_dst( (node_f @ W_side) * deg_out^-1/2 [src] )
Refactor used here (projection is linear, graph strictly bipartite):
    rst[d] = ( sum_{e->d} c_e * f_raw[src_e] ) @ W_side(d),
    c_e = deg_out[src]^-1/2 * deg_in[dst]^-1/2  (folded on host into scatter tiles)

Sharding: dst nodes dealt round-robin (degree-sorted) to 8 cores -> identical
compile-time schedule per core (SPMD), no collectives. Per core the device:
  1. dma_gather raw bf16 feature rows by src (256B rows). The gathers are
     spread across all 4 SWDGE queues: the Q7 ucode pins each queue's
     descriptor generation to one pair of GPSIMD cores (cpu_id/2 ==
     queue_num), so 4 in-flight gathers generate descriptors on 4 core
     pairs concurrently instead of serializing on pair 0.
  2. scatter-matmul (bf16): PSUM[128f, 512slot] += M_chunk[128e,128f].T
     @ S_chunk[128e,w] where S carries c_e at (edge_row, dst_col).
  3. projection matmul with the side weight (bf16 -> fp32 PSUM).
  4. feat-major fp32 output [128, slots]; host transposes/unpermutes.
"""
import sys
import os

for _p in ("/opt/trn_rl_repo",):
    if _p not in sys.path and os.path.isdir(_p):
        sys.path.insert(0, _p)

import numpy as np
import ml_dtypes

BF16 = ml_dtypes.bfloat16

N_U = 50000
N_V = 50000
N = N_U + N_V
D = 128
E = 1600000
N_CORES = 8
HALF = 25000          # int16-safe table window
WIN = 512             # dst slots per PSUM window
P = 128
NB = 4                # M/S bank depth (gather pipeline)
NQ = 4                # SWDGE queues (each owns one GPSIMD core pair)


# ----------------------------------------------------------------- host layout
def _build_layout(src, dst, cout, cin):
    """Canonical schedule + per-core edge/scatter data.

    Returns (schedule, per_core), where schedule is compile-time (identical
    across cores) and per_core holds idx/S arrays + output dst mapping.
    """
    layout_phases = []
    per_core_idx = [[] for _ in range(N_CORES)]
    per_core_sval = [[] for _ in range(N_CORES)]   # aligned with idx positions
    per_core_dsts = [[] for _ in range(N_CORES)]   # slot -> global dst id (-1 pad)

    for phase in range(2):
        if phase == 0:       # dsts are v-nodes, sources u-side
            mask = dst >= N_U
            d_local = dst[mask] - N_U
            s_local = src[mask]
            dst_base = N_U
        else:                # dsts are u-nodes, sources v-side
            mask = dst < N_U
            d_local = dst[mask]
            s_local = src[mask] - N_U
            dst_base = 0
        half = (s_local >= HALF).astype(np.int64)
        s_half_local = s_local - half * HALF

        n_dst = N_U
        a_cnt = np.bincount(d_local[half == 0], minlength=n_dst)
        b_cnt = np.bincount(d_local[half == 1], minlength=n_dst)

        order = np.lexsort((np.arange(n_dst), b_cnt, a_cnt))
        rank = np.empty(n_dst, np.int64)
        rank[order] = np.arange(n_dst)

        slots_per_core = (n_dst + N_CORES - 1) // N_CORES  # 6250
        # canonical per-slot degrees = max over cores (clipped >= 1)
        a_mat = np.zeros((N_CORES, slots_per_core), np.int64)
        b_mat = np.zeros((N_CORES, slots_per_core), np.int64)
        dst_mat = np.full((N_CORES, slots_per_core), -1, np.int64)
        r = np.arange(n_dst)
        a_mat[r % N_CORES, r // N_CORES] = a_cnt[order]
        b_mat[r % N_CORES, r // N_CORES] = b_cnt[order]
        dst_mat[r % N_CORES, r // N_CORES] = order + dst_base
        A = np.maximum(a_mat.max(axis=0), 1)
        B = np.maximum(b_mat.max(axis=0), 1)

        for k in range(N_CORES):
            per_core_dsts[k].append(dst_mat[k])

        # ---- canonical chunking per (window, pass), no slot straddles a chunk
        n_win = (slots_per_core + WIN - 1) // WIN
        windows = []
        # canonical edge-position base per slot, per pass
        pos_base = [np.zeros(slots_per_core, np.int64) for _ in (0, 1)]
        for w in range(n_win):
            s0, s1 = w * WIN, min((w + 1) * WIN, slots_per_core)
            wininfo = {"n_slots": s1 - s0, "passes": []}
            for p_i, C in enumerate((A, B)):
                chunks = []   # (col0, w, scol0)
                cur = 0       # fill in current chunk
                cur_chunk = None
                blocks = 0
                for s in range(s0, s1):
                    c = int(C[s])
                    if cur_chunk is None or cur + c > P:
                        if cur_chunk is not None:
                            chunks.append(cur_chunk)
                        cur_chunk = {"col0": s - s0, "cols": 0}
                        blocks += 1
                        cur = 0
                    pos_base[p_i][s] = (blocks - 1) * P + cur
                    cur += c
                    cur_chunk["cols"] = (s - s0) - cur_chunk["col0"] + 1
                if cur_chunk is not None:
                    chunks.append(cur_chunk)
                wininfo["passes"].append({"chunks": chunks, "n_blocks": blocks})
            windows.append(wininfo)
        layout_phases.append({
            "n_win": n_win,
            "slots_per_core": slots_per_core,
            "windows": windows,
        })

        # ---- per-core edge placement (vectorized)
        # rank within (dst, half) group:
        grp = d_local * 2 + half
        sort_i = np.argsort(grp, kind="stable")
        grp_s = grp[sort_i]
        starts = np.r_[0, np.nonzero(np.diff(grp_s))[0] + 1]
        group_id = np.cumsum(np.r_[0, (np.diff(grp_s) != 0).astype(np.int64)])
        first_pos_of_group = starts[group_id]
        within = np.arange(len(grp_s)) - first_pos_of_group
        e_rank = np.empty(len(grp), np.int64)
        e_rank[sort_i] = within

        e_core = rank[d_local] % N_CORES
        e_slot = rank[d_local] // N_CORES
        e_win = e_slot // WIN

        # global canonical position of each edge within its (win, pass) stream:
        e_pos = np.where(half == 0,
                         pos_base[0][e_slot],
                         pos_base[1][e_slot]) + e_rank

        # canonical call sizes (blocks) per (win, pass):
        call_blocks = np.array(
            [[windows[w]["passes"][p]["n_blocks"] for p in (0, 1)]
             for w in range(n_win)], np.int64)
        # canonical flat offsets: order = win-major, pass lo then hi
        call_sizes = (call_blocks * P).reshape(-1)           # [n_win*2]
        call_off = np.r_[0, np.cumsum(call_sizes)][:-1].reshape(n_win, 2)
        tot_idx = int(call_sizes.sum())

        # canonical S layout: per chunk scol0
        s_cols_per_call = []
        for w in range(n_win):
            for p_i in (0, 1):
                ch = windows[w]["passes"][p_i]["chunks"]
                cols = np.array([c["cols"] for c in ch], np.int64)
                s_cols_per_call.append(cols)
        chunk_cols_flat = np.concatenate(s_cols_per_call)
        chunk_scol0 = np.r_[0, np.cumsum(chunk_cols_flat)][:-1]
        tot_scols = int(chunk_cols_flat.sum())
        # record scol0 / col0 back into schedule for device build
        # (scol0 made global across phases via scol_phase_base)
        scol_phase_base = sum(
            pc.shape[1] for pc in per_core_sval[0]
        ) if per_core_sval[0] else 0
        ci = 0
        for w in range(n_win):
            for p_i in (0, 1):
                for c in windows[w]["passes"][p_i]["chunks"]:
                    c["scol0"] = int(chunk_scol0[ci]) + scol_phase_base
                    ci += 1

        # per-chunk col0 arrays for edge->scol math
        chunk_col0_flat = np.concatenate(
            [np.array([c["col0"] for c in windows[w]["passes"][p_i]["chunks"]],
                      np.int64)
             for w in range(n_win) for p_i in (0, 1)])
        # chunk global id for an edge: need per-call chunk base
        chunks_per_call = np.array([len(s) for s in s_cols_per_call], np.int64)
        call_chunk_base = np.r_[0, np.cumsum(chunks_per_call)][:-1].reshape(n_win, 2)

        e_call_off = call_off[e_win, half]
        e_gpos = e_call_off + e_pos                      # global idx position
        e_chunk = call_chunk_base[e_win, half] + e_pos // P
        e_row = e_pos % P
        e_scol = chunk_scol0[e_chunk] + (e_slot - e_win * WIN) - chunk_col0_flat[e_chunk]

        e_val = (cout[s_local + (0 if phase == 0 else N_U)]
                 * cin[d_local + dst_base]).astype(np.float32)

        for k in range(N_CORES):
            m = e_core == k
            idx_flat = np.zeros(tot_idx, np.int16)
            idx_flat[e_gpos[m]] = s_half_local[m].astype(np.int16)
            sv = np.zeros((P, tot_scols), BF16)
            sv[e_row[m], e_scol[m]] = e_val[m].astype(BF16)
            per_core_idx[k].append(idx_flat)
            per_core_sval[k].append(sv)

    # wrap idx per call into the [16, n/16].T-tiled layout, concat everything
    per_core = []
    for k in range(N_CORES):
        idx_cols = []
        for phase in range(2):
            ph = layout_phases[phase]
            flat = per_core_idx[k][phase]
            off = 0
            for w in range(ph["n_win"]):
                for p_i in (0, 1):
                    nb = ph["windows"][w]["passes"][p_i]["n_blocks"]
                    n = nb * P
                    call = flat[off:off + n]
                    off += n
                    t = call.reshape(n // 16, 16).T      # [16, n/16]
                    idx_cols.append(np.tile(t, (N_CORES, 1)))
        idx_arr = np.concatenate(idx_cols, axis=1)       # [128, tot/16]
        s_arr = np.concatenate(per_core_sval[k], axis=1)  # [128, scols]
        per_core.append({"idx": idx_arr, "s": s_arr, "dsts": per_core_dsts[k]})
    return layout_phases, per_core


# ------------------------------------------------------------------ device code
def _build_nc(sched):
    import concourse.bacc as bacc
    import concourse.bass as bass
    import concourse.mybir as mybir
    from concourse._compat import get_trn_type
    from concourse.library_config import mlp

    nc = bacc.Bacc(get_trn_type() or "TRN2", target_bir_lowering=False,
                   debug=False, num_swdge_queues=NQ)
    f32 = mybir.dt.float32
    bf16 = mybir.dt.bfloat16
    u_f = nc.dram_tensor("u_f", [N_U, D], bf16, kind="ExternalInput")
    v_f = nc.dram_tensor("v_f", [N_V, D], bf16, kind="ExternalInput")
    u_w = nc.dram_tensor("u_w", [D, D], bf16, kind="ExternalInput")
    v_w = nc.dram_tensor("v_w", [D, D], bf16, kind="ExternalInput")

    # totals from schedule
    tot_idx_cols = 0
    tot_scols = 0
    tot_slots = 0
    nblk_max = 0
    for ph in sched:
        for w in ph["windows"]:
            tot_slots += w["n_slots"]
            nb = 0
            for p_i in (0, 1):
                pa = w["passes"][p_i]
                nb += pa["n_blocks"]
                tot_idx_cols += pa["n_blocks"] * P // 16
                tot_scols += sum(c["cols"] for c in pa["chunks"])
            nblk_max = max(nblk_max, nb)

    idx_in = nc.dram_tensor("idx", [P, tot_idx_cols], mybir.dt.int16,
                            kind="ExternalInput")
    s_in = nc.dram_tensor("sval", [P, tot_scols], bf16, kind="ExternalInput")
    out = nc.dram_tensor("out", [P, tot_slots], bf16, kind="ExternalOutput")

    idx_sb = nc.alloc_sbuf_tensor("idx_sb", [P, tot_idx_cols], mybir.dt.int16)
    m_sb = [nc.alloc_sbuf_tensor(f"m{i}", [P, nblk_max, P], bf16)
            for i in range(NB)]
    s_sb = [nc.alloc_sbuf_tensor(f"s{i}", [P, 2 * WIN], bf16)
            for i in range(NB)]
    agg_sb = [nc.alloc_sbuf_tensor(f"agg{i}", [P, WIN], bf16) for i in (0, 1)]
    stage_sb = nc.alloc_sbuf_tensor("stage", [P, tot_slots], bf16)
    w_sb = [nc.alloc_sbuf_tensor(f"w{i}", [P, D], bf16) for i in (0, 1)]

    agg_ps = [nc.alloc_psum_tensor(f"aps{i}", [P, WIN], f32) for i in (0, 1)]
    proj_ps = [nc.alloc_psum_tensor(f"pps{i}", [P, WIN], f32) for i in (0, 1)]

    sem_ld = nc.alloc_semaphore("ld")        # upfront loads + final store
    sem_idx = nc.alloc_semaphore("idxld")    # idx table load
    sem_s = [nc.alloc_semaphore(f"ssem{i}") for i in range(NB)]
    sem_g = [nc.alloc_semaphore(f"gsem{i}") for i in range(NB)]
    sem_mm = [nc.alloc_semaphore(f"mmsem{i}") for i in range(NB)]
    sem_agg = [nc.alloc_semaphore(f"aggsem{i}") for i in (0, 1)]
    sem_proj = [nc.alloc_semaphore(f"projsem{i}") for i in (0, 1)]
    sem_stage = [nc.alloc_semaphore(f"stsem{i}") for i in (0, 1)]

    # flatten windows across phases into one global list
    wlist = []
    icol = 0
    scol = 0
    slot0 = 0
    for phase, ph in enumerate(sched):
        for w in ph["windows"]:
            entry = {"phase": phase, "n_slots": w["n_slots"], "passes": [],
                     "slot0": slot0}
            for p_i in (0, 1):
                pa = w["passes"][p_i]
                nb = pa["n_blocks"]
                entry["passes"].append({
                    "icol": icol, "nb": nb,
                    "chunks": pa["chunks"], "scol": scol,
                })
                icol += nb * P // 16
                scol += sum(c["cols"] for c in pa["chunks"])
            slot0 += w["n_slots"]
            wlist.append(entry)
    NW = len(wlist)
    n_even = (NW + 1) // 2
    n_odd = NW // 2

    # Balanced gather sub-calls: 4 per window (one per SWDGE queue), sizes
    # as equal as possible so the 4 Q7 core pairs finish together (in-order
    # completion with ~4 outstanding Pool instructions locksteps each
    # quad on its slowest member).
    def _split(n, k):
        q, r = divmod(n, k)
        return [q + (i < r) for i in range(k)]

    for wi, went in enumerate(wlist):
        nb_lo = went["passes"][0]["nb"]
        nb_hi = went["passes"][1]["nb"]
        tot = nb_lo + nb_hi
        if nb_lo == 0:
            a = 0
        elif nb_hi == 0:
            a = 4
        else:
            a = min(3, max(1, round(4 * nb_lo / tot)))
        subs = []   # (queue, p_i, blk_s, blk_e)
        qi = 0
        for p_i, npart, nb in ((0, a, nb_lo), (1, 4 - a, nb_hi)):
            if npart == 0 or nb == 0:
                continue
            s = 0
            for sz in _split(nb, npart):
                if sz:
                    # rotate queue assignment per window so any systematic
                    # per-queue cost is spread evenly across the 4 lanes
                    subs.append(((qi + wi) % NQ, p_i, s, s + sz))
                    qi += 1
                    s += sz
        went["subs"] = subs

    # idx preload split point (after window 3) for a faster pipeline start
    idx_c0 = wlist[4]["passes"][0]["icol"] if NW > 4 else tot_idx_cols
    # output stream pieces: after these windows, flush finished stage slots
    flush_after = sorted(set([NW // 4 - 1, NW // 2 - 1, 3 * NW // 4 - 1, NW - 1]))
    flush_after = [f for f in flush_after if f >= 0]

    with nc.Block() as block:
        @block.sync
        def _(sy: bass.BassEngine):
            sy.dma_start(idx_sb[:, :idx_c0], idx_in[:, :idx_c0]).then_inc(
                sem_idx, 16)
            sy.dma_start(w_sb[0][:], u_w[:]).then_inc(sem_ld, 16)
            sy.dma_start(w_sb[1][:], v_w[:]).then_inc(sem_ld, 16)
            if idx_c0 < tot_idx_cols:
                sy.dma_start(idx_sb[:, idx_c0:], idx_in[:, idx_c0:]).then_inc(
                    sem_idx, 16)
            n_flush = 0
            flush_slot0 = 0
            for wi, went in enumerate(wlist):
                b = wi % NB
                # WAR: S bank b free after window wi-NB's matmuls done
                if wi >= NB:
                    sy.wait_ge(sem_mm[b], wi // NB)
                p0, p1 = went["passes"]
                ncols = (sum(c["cols"] for c in p0["chunks"])
                         + sum(c["cols"] for c in p1["chunks"]))
                sy.dma_start(
                    s_sb[b][:, :ncols], s_in[:, p0["scol"]:p0["scol"] + ncols]
                ).then_inc(sem_s[b], 16)
                if wi in flush_after:
                    # stream finished stage slots out while later windows run
                    sy.wait_ge(sem_stage[0], wi // 2 + 1)
                    sy.wait_ge(sem_stage[1], (wi + 1) // 2)
                    end = went["slot0"] + went["n_slots"]
                    sy.dma_start(
                        out[:, flush_slot0:end], stage_sb[:, flush_slot0:end]
                    ).then_inc(sem_ld, 16)
                    flush_slot0 = end
                    n_flush += 1
            sy.wait_ge(sem_ld, 32 + 16 * n_flush)

        @block.gpsimd
        def _(gp: bass.BassGpSimd):
            gp.load_library(mlp)
            for wi, went in enumerate(wlist):
                b = wi % NB
                phase = went["phase"]
                if phase == 0:
                    tab_lo, tab_hi = u_f[0:HALF, :], u_f[HALF:N_U, :]
                else:
                    tab_lo, tab_hi = v_f[0:HALF, :], v_f[HALF:N_V, :]
                nb_lo = went["passes"][0]["nb"]
                if wi == 0:
                    gp.wait_ge(sem_idx, 16)
                if wi == 4 and idx_c0 < tot_idx_cols:
                    gp.wait_ge(sem_idx, 32)
                if wi >= NB:
                    gp.wait_ge(sem_mm[b], wi // NB)
                for q, p_i, blk_s, blk_e in went["subs"]:
                    pa = went["passes"][p_i]
                    tab = tab_lo if p_i == 0 else tab_hi
                    nsub = (blk_e - blk_s) * P
                    blk_base = (0 if p_i == 0 else nb_lo) + blk_s
                    icol_s = pa["icol"] + blk_s * P // 16
                    gp.dma_gather(
                        m_sb[b][:, blk_base:blk_base + (blk_e - blk_s), :],
                        tab,
                        idx_sb[:, icol_s:icol_s + nsub // 16],
                        nsub, nsub, D,
                        single_packet=False,
                        queue_num=q,
                    ).then_inc(sem_g[b], 16)

        @block.tensor
        def _(te):
            g_seen = [0] * NB
            s_seen = [0] * NB
            for wi, went in enumerate(wlist):
                b = wi % NB
                pb = wi % 2
                phase = went["phase"]
                # all gather sub-calls + S stream for this window
                g_seen[b] += 16 * len(went["subs"])
                s_seen[b] += 16
                te.wait_ge(sem_g[b], g_seen[b])
                te.wait_ge(sem_s[b], s_seen[b])
                # agg_ps WAR vs vector copy of window wi-2
                if wi >= 2:
                    te.wait_ge(sem_agg[pb], wi // 2)
                ns = went["n_slots"]
                blk0 = 0
                last = None
                for p_i in (0, 1):
                    pa = went["passes"][p_i]
                    for ci, ch in enumerate(pa["chunks"]):
                        last = (p_i, ci)
                first = True
                for p_i in (0, 1):
                    pa = went["passes"][p_i]
                    for ci, ch in enumerate(pa["chunks"]):
                        blk = blk0 + ci
                        sc = ch["scol0"] - went["passes"][0]["scol"]
                        mm = te.matmul(
                            out=agg_ps[pb][:, ch["col0"]:ch["col0"] + ch["cols"]],
                            lhsT=m_sb[b][:, blk, :],
                            rhs=s_sb[b][:, sc:sc + ch["cols"]],
                            start=first,
                            stop=((p_i, ci) == last),
                        )
                        first = False
                        if (p_i, ci) == last:
                            mm.then_inc(sem_mm[b], 1)
                    blk0 += pa["nb"]
                # projection: wait for vector to copy agg->sbuf (this window)
                te.wait_ge(sem_agg[pb], wi // 2 + 1)
                # proj_ps WAR vs vector stage copy of window wi-2
                if wi >= 2:
                    te.wait_ge(sem_stage[pb], wi // 2)
                te.matmul(
                    out=proj_ps[pb][:, :ns],
                    lhsT=w_sb[phase][:],
                    rhs=agg_sb[pb][:, :ns],
                    start=True, stop=True,
                ).then_inc(sem_proj[pb], 1)

        @block.vector
        def _(ve):
            for wi, went in enumerate(wlist):
                b = wi % NB
                pb = wi % 2
                ns = went["n_slots"]
                ve.wait_ge(sem_mm[b], wi // NB + 1)
                ve.tensor_copy(out=agg_sb[pb][:, :ns],
                               in_=agg_ps[pb][:, :ns]).then_inc(sem_agg[pb], 1)
                ve.wait_ge(sem_proj[pb], wi // 2 + 1)
                ve.tensor_copy(
                    out=stage_sb[:, went["slot0"]:went["slot0"] + ns],
                    in_=proj_ps[pb][:, :ns],
                ).then_inc(sem_stage[pb], 1)

    nc.compile()
    return nc


# ---------------------------------------------------------------------- kernel
def kernel(u_f, v_f, u_w, v_w, src, dst):
    from concourse.bass_utils import run_bass_kernel_spmd

    src = np.asarray(src)
    dst = np.asarray(dst)
    u_f = np.asarray(u_f, np.float32)
    v_f = np.asarray(v_f, np.float32)

    deg_out = np.bincount(src, minlength=N).astype(np.float32)
    deg_in = np.bincount(dst, minlength=N).astype(np.float32)
    cout = np.maximum(deg_out, 1.0) ** -0.5
    cin = np.maximum(deg_in, 1.0) ** -0.5

    sched, per_core = _build_layout(src, dst, cout, cin)

    nc = _build_nc(sched)
    u_f_b = u_f.astype(BF16)
    v_f_b = v_f.astype(BF16)
    u_w_b = np.asarray(u_w, np.float32).astype(BF16)
    v_w_b = np.asarray(v_w, np.float32).astype(BF16)
    in_maps = []
    for k in range(N_CORES):
        in_maps.append({
            "u_f": u_f_b, "v_f": v_f_b,
            "u_w": u_w_b, "v_w": v_w_b,
            "idx": per_core[k]["idx"], "sval": per_core[k]["s"],
        })
    trace = bool(os.environ.get("KERNEL_TRACE"))
    res = run_bass_kernel_spmd(nc, in_maps, core_ids=list(range(N_CORES)),
                               trace=trace)
    if trace:
        print(f"HW exec time: {res.exec_time_ns} ns")
        kernel.last_profile = res.profile_json

    out_full = np.zeros((N, D), np.float32)
    for k in range(N_CORES):
        fm = np.asarray(res.results[k]["out"], dtype=np.float32)
        rows = np.ascontiguousarray(fm.T)     # [tot_slots, 128]
        slot0 = 0
        for phase in range(2):
            dsts = per_core[k]["dsts"][phase]
            nslots = len(dsts)
            valid = dsts >= 0
            out_full[dsts[valid]] = rows[slot0:slot0 + nslots][valid]
            slot0 += nslots
    return out_full


# revision 16
# speedup vs baseline: 1.7097x; 1.0066x over previous
"""Bipartite GCN message-passing kernel for 8 Trainium2 NeuronCores.

Math (reference): rst = deg_in^-1/2 * segsum_dst( (node_f @ W_side) * deg_out^-1/2 [src] )
Refactor used here (projection is linear, graph strictly bipartite):
    rst[d] = ( sum_{e->d} c_e * f_raw[src_e] ) @ W_side(d),
    c_e = deg_out[src]^-1/2 * deg_in[dst]^-1/2  (folded on host into scatter tiles)

Sharding: dst nodes dealt round-robin (degree-sorted) to 8 cores -> identical
compile-time schedule per core (SPMD), no collectives. Per core the device:
  1. dma_gather raw bf16 feature rows by src (256B rows). The gathers are
     spread across all 4 SWDGE queues: the Q7 ucode pins each queue's
     descriptor generation to one pair of GPSIMD cores (cpu_id/2 ==
     queue_num), so 4 in-flight gathers generate descriptors on 4 core
     pairs concurrently instead of serializing on pair 0.
  2. scatter-matmul (bf16): PSUM[128f, 512slot] += M_chunk[128e,128f].T
     @ S_chunk[128e,w] where S carries c_e at (edge_row, dst_col).
  3. projection matmul with the side weight (bf16 -> fp32 PSUM).
  4. feat-major fp32 output [128, slots]; host transposes/unpermutes.
"""
import sys
import os

for _p in ("/opt/trn_rl_repo",):
    if _p not in sys.path and os.path.isdir(_p):
        sys.path.insert(0, _p)

import numpy as np
import ml_dtypes

BF16 = ml_dtypes.bfloat16

N_U = 50000
N_V = 50000
N = N_U + N_V
D = 128
E = 1600000
N_CORES = 8
HALF = 25000          # int16-safe table window
WIN = 512             # dst slots per PSUM window
P = 128
NB = 4                # M/S bank depth (gather pipeline)
NQ = 4                # SWDGE queues (each owns one GPSIMD core pair)


# ----------------------------------------------------------------- host layout
def _build_layout(src, dst, cout, cin):
    """Canonical schedule + per-core edge/scatter data.

    Returns (schedule, per_core), where schedule is compile-time (identical
    across cores) and per_core holds idx/S arrays + output dst mapping.
    """
    layout_phases = []
    per_core_idx = [[] for _ in range(N_CORES)]
    per_core_sval = [[] for _ in range(N_CORES)]   # aligned with idx positions
    per_core_dsts = [[] for _ in range(N_CORES)]   # slot -> global dst id (-1 pad)

    for phase in range(2):
        if phase == 0:       # dsts are v-nodes, sources u-side
            mask = dst >= N_U
            d_local = dst[mask] - N_U
            s_local = src[mask]
            dst_base = N_U
        else:                # dsts are u-nodes, sources v-side
            mask = dst < N_U
            d_local = dst[mask]
            s_local = src[mask] - N_U
            dst_base = 0
        half = (s_local >= HALF).astype(np.int64)
        s_half_local = s_local - half * HALF

        n_dst = N_U
        a_cnt = np.bincount(d_local[half == 0], minlength=n_dst)
        b_cnt = np.bincount(d_local[half == 1], minlength=n_dst)

        order = np.lexsort((np.arange(n_dst), b_cnt, a_cnt))
        rank = np.empty(n_dst, np.int64)
        rank[order] = np.arange(n_dst)

        slots_per_core = (n_dst + N_CORES - 1) // N_CORES  # 6250
        # canonical per-slot degrees = max over cores (clipped >= 1)
        a_mat = np.zeros((N_CORES, slots_per_core), np.int64)
        b_mat = np.zeros((N_CORES, slots_per_core), np.int64)
        dst_mat = np.full((N_CORES, slots_per_core), -1, np.int64)
        r = np.arange(n_dst)
        a_mat[r % N_CORES, r // N_CORES] = a_cnt[order]
        b_mat[r % N_CORES, r // N_CORES] = b_cnt[order]
        dst_mat[r % N_CORES, r // N_CORES] = order + dst_base
        A = np.maximum(a_mat.max(axis=0), 1)
        B = np.maximum(b_mat.max(axis=0), 1)

        for k in range(N_CORES):
            per_core_dsts[k].append(dst_mat[k])

        # ---- canonical chunking per (window, pass), no slot straddles a chunk
        n_win = (slots_per_core + WIN - 1) // WIN
        windows = []
        # canonical edge-position base per slot, per pass
        pos_base = [np.zeros(slots_per_core, np.int64) for _ in (0, 1)]
        for w in range(n_win):
            s0, s1 = w * WIN, min((w + 1) * WIN, slots_per_core)
            wininfo = {"n_slots": s1 - s0, "passes": []}
            for p_i, C in enumerate((A, B)):
                chunks = []   # (col0, w, scol0)
                cur = 0       # fill in current chunk
                cur_chunk = None
                blocks = 0
                for s in range(s0, s1):
                    c = int(C[s])
                    if cur_chunk is None or cur + c > P:
                        if cur_chunk is not None:
                            chunks.append(cur_chunk)
                        cur_chunk = {"col0": s - s0, "cols": 0}
                        blocks += 1
                        cur = 0
                    pos_base[p_i][s] = (blocks - 1) * P + cur
                    cur += c
                    cur_chunk["cols"] = (s - s0) - cur_chunk["col0"] + 1
                if cur_chunk is not None:
                    chunks.append(cur_chunk)
                wininfo["passes"].append({"chunks": chunks, "n_blocks": blocks})
            windows.append(wininfo)
        layout_phases.append({
            "n_win": n_win,
            "slots_per_core": slots_per_core,
            "windows": windows,
        })

        # ---- per-core edge placement (vectorized)
        # rank within (dst, half) group:
        grp = d_local * 2 + half
        sort_i = np.argsort(grp, kind="stable")
        grp_s = grp[sort_i]
        starts = np.r_[0, np.nonzero(np.diff(grp_s))[0] + 1]
        group_id = np.cumsum(np.r_[0, (np.diff(grp_s) != 0).astype(np.int64)])
        first_pos_of_group = starts[group_id]
        within = np.arange(len(grp_s)) - first_pos_of_group
        e_rank = np.empty(len(grp), np.int64)
        e_rank[sort_i] = within

        e_core = rank[d_local] % N_CORES
        e_slot = rank[d_local] // N_CORES
        e_win = e_slot // WIN

        # global canonical position of each edge within its (win, pass) stream:
        e_pos = np.where(half == 0,
                         pos_base[0][e_slot],
                         pos_base[1][e_slot]) + e_rank

        # canonical call sizes (blocks) per (win, pass):
        call_blocks = np.array(
            [[windows[w]["passes"][p]["n_blocks"] for p in (0, 1)]
             for w in range(n_win)], np.int64)
        # canonical flat offsets: order = win-major, pass lo then hi
        call_sizes = (call_blocks * P).reshape(-1)           # [n_win*2]
        call_off = np.r_[0, np.cumsum(call_sizes)][:-1].reshape(n_win, 2)
        tot_idx = int(call_sizes.sum())

        # canonical S layout: per chunk scol0
        s_cols_per_call = []
        for w in range(n_win):
            for p_i in (0, 1):
                ch = windows[w]["passes"][p_i]["chunks"]
                cols = np.array([c["cols"] for c in ch], np.int64)
                s_cols_per_call.append(cols)
        chunk_cols_flat = np.concatenate(s_cols_per_call)
        chunk_scol0 = np.r_[0, np.cumsum(chunk_cols_flat)][:-1]
        tot_scols = int(chunk_cols_flat.sum())
        # record scol0 / col0 back into schedule for device build
        # (scol0 made global across phases via scol_phase_base)
        scol_phase_base = sum(
            pc.shape[1] for pc in per_core_sval[0]
        ) if per_core_sval[0] else 0
        ci = 0
        for w in range(n_win):
            for p_i in (0, 1):
                for c in windows[w]["passes"][p_i]["chunks"]:
                    c["scol0"] = int(chunk_scol0[ci]) + scol_phase_base
                    ci += 1

        # per-chunk col0 arrays for edge->scol math
        chunk_col0_flat = np.concatenate(
            [np.array([c["col0"] for c in windows[w]["passes"][p_i]["chunks"]],
                      np.int64)
             for w in range(n_win) for p_i in (0, 1)])
        # chunk global id for an edge: need per-call chunk base
        chunks_per_call = np.array([len(s) for s in s_cols_per_call], np.int64)
        call_chunk_base = np.r_[0, np.cumsum(chunks_per_call)][:-1].reshape(n_win, 2)

        e_call_off = call_off[e_win, half]
        e_gpos = e_call_off + e_pos                      # global idx position
        e_chunk = call_chunk_base[e_win, half] + e_pos // P
        e_row = e_pos % P
        e_scol = chunk_scol0[e_chunk] + (e_slot - e_win * WIN) - chunk_col0_flat[e_chunk]

        e_val = (cout[s_local + (0 if phase == 0 else N_U)]
                 * cin[d_local + dst_base]).astype(np.float32)

        for k in range(N_CORES):
            m = e_core == k
            idx_flat = np.zeros(tot_idx, np.int16)
            idx_flat[e_gpos[m]] = s_half_local[m].astype(np.int16)
            sv = np.zeros((P, tot_scols), BF16)
            sv[e_row[m], e_scol[m]] = e_val[m].astype(BF16)
            per_core_idx[k].append(idx_flat)
            per_core_sval[k].append(sv)

    # wrap idx per call into the [16, n/16].T-tiled layout, concat everything
    per_core = []
    for k in range(N_CORES):
        idx_cols = []
        for phase in range(2):
            ph = layout_phases[phase]
            flat = per_core_idx[k][phase]
            off = 0
            for w in range(ph["n_win"]):
                for p_i in (0, 1):
                    nb = ph["windows"][w]["passes"][p_i]["n_blocks"]
                    n = nb * P
                    call = flat[off:off + n]
                    off += n
                    t = call.reshape(n // 16, 16).T      # [16, n/16]
                    idx_cols.append(np.tile(t, (N_CORES, 1)))
        idx_arr = np.concatenate(idx_cols, axis=1)       # [128, tot/16]
        s_arr = np.concatenate(per_core_sval[k], axis=1)  # [128, scols]
        per_core.append({"idx": idx_arr, "s": s_arr, "dsts": per_core_dsts[k]})
    return layout_phases, per_core


# ------------------------------------------------------------------ device code
def _build_nc(sched):
    import concourse.bacc as bacc
    import concourse.bass as bass
    import concourse.mybir as mybir
    from concourse._compat import get_trn_type
    from concourse.library_config import mlp

    nc = bacc.Bacc(get_trn_type() or "TRN2", target_bir_lowering=False,
                   debug=False, num_swdge_queues=NQ)
    f32 = mybir.dt.float32
    bf16 = mybir.dt.bfloat16
    u_f = nc.dram_tensor("u_f", [N_U, D], bf16, kind="ExternalInput")
    v_f = nc.dram_tensor("v_f", [N_V, D], bf16, kind="ExternalInput")
    u_w = nc.dram_tensor("u_w", [D, D], bf16, kind="ExternalInput")
    v_w = nc.dram_tensor("v_w", [D, D], bf16, kind="ExternalInput")

    # totals from schedule
    tot_idx_cols = 0
    tot_scols = 0
    tot_slots = 0
    nblk_max = 0
    for ph in sched:
        for w in ph["windows"]:
            tot_slots += w["n_slots"]
            nb = 0
            for p_i in (0, 1):
                pa = w["passes"][p_i]
                nb += pa["n_blocks"]
                tot_idx_cols += pa["n_blocks"] * P // 16
                tot_scols += sum(c["cols"] for c in pa["chunks"])
            nblk_max = max(nblk_max, nb)

    idx_in = nc.dram_tensor("idx", [P, tot_idx_cols], mybir.dt.int16,
                            kind="ExternalInput")
    s_in = nc.dram_tensor("sval", [P, tot_scols], bf16, kind="ExternalInput")
    out = nc.dram_tensor("out", [P, tot_slots], bf16, kind="ExternalOutput")

    idx_sb = nc.alloc_sbuf_tensor("idx_sb", [P, tot_idx_cols], mybir.dt.int16)
    m_sb = [nc.alloc_sbuf_tensor(f"m{i}", [P, nblk_max, P], bf16)
            for i in range(NB)]
    s_sb = [nc.alloc_sbuf_tensor(f"s{i}", [P, 2 * WIN], bf16)
            for i in range(NB)]
    agg_sb = [nc.alloc_sbuf_tensor(f"agg{i}", [P, WIN], bf16) for i in (0, 1)]
    stage_sb = nc.alloc_sbuf_tensor("stage", [P, tot_slots], bf16)
    w_sb = [nc.alloc_sbuf_tensor(f"w{i}", [P, D], bf16) for i in (0, 1)]

    agg_ps = [nc.alloc_psum_tensor(f"aps{i}", [P, WIN], f32) for i in (0, 1)]
    proj_ps = [nc.alloc_psum_tensor(f"pps{i}", [P, WIN], f32) for i in (0, 1)]

    sem_ld = nc.alloc_semaphore("ld")        # upfront loads + final store
    sem_idx = nc.alloc_semaphore("idxld")    # idx table load
    sem_s = [nc.alloc_semaphore(f"ssem{i}") for i in range(NB)]
    sem_g = [nc.alloc_semaphore(f"gsem{i}") for i in range(NB)]
    sem_mm = [nc.alloc_semaphore(f"mmsem{i}") for i in range(NB)]
    sem_agg = [nc.alloc_semaphore(f"aggsem{i}") for i in (0, 1)]
    sem_proj = [nc.alloc_semaphore(f"projsem{i}") for i in (0, 1)]
    sem_stage = [nc.alloc_semaphore(f"stsem{i}") for i in (0, 1)]

    # flatten windows across phases into one global list
    wlist = []
    icol = 0
    scol = 0
    slot0 = 0
    for phase, ph in enumerate(sched):
        for w in ph["windows"]:
            entry = {"phase": phase, "n_slots": w["n_slots"], "passes": [],
                     "slot0": slot0}
            for p_i in (0, 1):
                pa = w["passes"][p_i]
                nb = pa["n_blocks"]
                entry["passes"].append({
                    "icol": icol, "nb": nb,
                    "chunks": pa["chunks"], "scol": scol,
                })
                icol += nb * P // 16
                scol += sum(c["cols"] for c in pa["chunks"])
            slot0 += w["n_slots"]
            wlist.append(entry)
    NW = len(wlist)
    n_even = (NW + 1) // 2
    n_odd = NW // 2

    # Balanced gather sub-calls: 4 per window (one per SWDGE queue), sizes
    # as equal as possible so the 4 Q7 core pairs finish together (in-order
    # completion with ~4 outstanding Pool instructions locksteps each
    # quad on its slowest member).
    def _split(n, k):
        q, r = divmod(n, k)
        return [q + (i < r) for i in range(k)]

    for wi, went in enumerate(wlist):
        nb_lo = went["passes"][0]["nb"]
        nb_hi = went["passes"][1]["nb"]
        tot = nb_lo + nb_hi
        if nb_lo == 0:
            a = 0
        elif nb_hi == 0:
            a = 4
        else:
            a = min(3, max(1, round(4 * nb_lo / tot)))
        subs = []   # (queue, p_i, blk_s, blk_e)
        qi = 0
        for p_i, npart, nb in ((0, a, nb_lo), (1, 4 - a, nb_hi)):
            if npart == 0 or nb == 0:
                continue
            s = 0
            for sz in _split(nb, npart):
                if sz:
                    # rotate queue assignment per window so any systematic
                    # per-queue cost is spread evenly across the 4 lanes
                    subs.append(((qi + wi) % NQ, p_i, s, s + sz))
                    qi += 1
                    s += sz
        went["subs"] = subs

    # idx preload split point (after window 3) for a faster pipeline start
    idx_c0 = wlist[4]["passes"][0]["icol"] if NW > 4 else tot_idx_cols
    # output stream pieces: after these windows, flush finished stage slots
    flush_after = sorted(set([NW // 4 - 1, NW // 2 - 1, 3 * NW // 4 - 1, NW - 1]))
    flush_after = [f for f in flush_after if f >= 0]

    with nc.Block() as block:
        @block.sync
        def _(sy: bass.BassEngine):
            sy.dma_start(idx_sb[:, :idx_c0], idx_in[:, :idx_c0]).then_inc(
                sem_idx, 16)
            sy.dma_start(w_sb[0][:], u_w[:]).then_inc(sem_ld, 16)
            sy.dma_start(w_sb[1][:], v_w[:]).then_inc(sem_ld, 16)
            if idx_c0 < tot_idx_cols:
                sy.dma_start(idx_sb[:, idx_c0:], idx_in[:, idx_c0:]).then_inc(
                    sem_idx, 16)
            n_flush = 0
            flush_slot0 = 0
            for wi, went in enumerate(wlist):
                b = wi % NB
                # WAR: S bank b free after window wi-NB's matmuls done
                if wi >= NB:
                    sy.wait_ge(sem_mm[b], wi // NB)
                p0, p1 = went["passes"]
                ncols = (sum(c["cols"] for c in p0["chunks"])
                         + sum(c["cols"] for c in p1["chunks"]))
                sy.dma_start(
                    s_sb[b][:, :ncols], s_in[:, p0["scol"]:p0["scol"] + ncols]
                ).then_inc(sem_s[b], 16)
                if wi in flush_after:
                    # stream finished stage slots out while later windows run
                    sy.wait_ge(sem_stage[0], wi // 2 + 1)
                    sy.wait_ge(sem_stage[1], (wi + 1) // 2)
                    end = went["slot0"] + went["n_slots"]
                    sy.dma_start(
                        out[:, flush_slot0:end], stage_sb[:, flush_slot0:end]
                    ).then_inc(sem_ld, 16)
                    flush_slot0 = end
                    n_flush += 1
            sy.wait_ge(sem_ld, 32 + 16 * n_flush)

        @block.gpsimd
        def _(gp: bass.BassGpSimd):
            gp.load_library(mlp)
            for wi, went in enumerate(wlist):
                b = wi % NB
                phase = went["phase"]
                if phase == 0:
                    tab_lo, tab_hi = u_f[0:HALF, :], u_f[HALF:N_U, :]
                else:
                    tab_lo, tab_hi = v_f[0:HALF, :], v_f[HALF:N_V, :]
                nb_lo = went["passes"][0]["nb"]
                if wi == 0:
                    gp.wait_ge(sem_idx, 16)
                if wi == 4 and idx_c0 < tot_idx_cols:
                    gp.wait_ge(sem_idx, 32)
                if wi >= NB:
                    gp.wait_ge(sem_mm[b], wi // NB)
                for q, p_i, blk_s, blk_e in went["subs"]:
                    pa = went["passes"][p_i]
                    tab = tab_lo if p_i == 0 else tab_hi
                    nsub = (blk_e - blk_s) * P
                    blk_base = (0 if p_i == 0 else nb_lo) + blk_s
                    icol_s = pa["icol"] + blk_s * P // 16
                    gp.dma_gather(
                        m_sb[b][:, blk_base:blk_base + (blk_e - blk_s), :],
                        tab,
                        idx_sb[:, icol_s:icol_s + nsub // 16],
                        nsub, nsub, D,
                        single_packet=False,
                        queue_num=q,
                    ).then_inc(sem_g[b], 16)

        @block.tensor
        def _(te):
            g_seen = [0] * NB
            s_seen = [0] * NB
            for wi, went in enumerate(wlist):
                b = wi % NB
                pb = wi % 2
                phase = went["phase"]
                # all gather sub-calls + S stream for this window
                g_seen[b] += 16 * len(went["subs"])
                s_seen[b] += 16
                te.wait_ge(sem_g[b], g_seen[b])
                te.wait_ge(sem_s[b], s_seen[b])
                # agg_ps WAR vs vector copy of window wi-2
                if wi >= 2:
                    te.wait_ge(sem_agg[pb], wi // 2)
                ns = went["n_slots"]
                blk0 = 0
                last = None
                for p_i in (0, 1):
                    pa = went["passes"][p_i]
                    for ci, ch in enumerate(pa["chunks"]):
                        last = (p_i, ci)
                first = True
                for p_i in (0, 1):
                    pa = went["passes"][p_i]
                    for ci, ch in enumerate(pa["chunks"]):
                        blk = blk0 + ci
                        sc = ch["scol0"] - went["passes"][0]["scol"]
                        mm = te.matmul(
                            out=agg_ps[pb][:, ch["col0"]:ch["col0"] + ch["cols"]],
                            lhsT=m_sb[b][:, blk, :],
                            rhs=s_sb[b][:, sc:sc + ch["cols"]],
                            start=first,
                            stop=((p_i, ci) == last),
                        )
                        first = False
                        if (p_i, ci) == last:
                            mm.then_inc(sem_mm[b], 1)
                    blk0 += pa["nb"]
                # projection: wait for vector to copy agg->sbuf (this window)
                te.wait_ge(sem_agg[pb], wi // 2 + 1)
                # proj_ps WAR vs vector stage copy of window wi-2
                if wi >= 2:
                    te.wait_ge(sem_stage[pb], wi // 2)
                te.matmul(
                    out=proj_ps[pb][:, :ns],
                    lhsT=w_sb[phase][:],
                    rhs=agg_sb[pb][:, :ns],
                    start=True, stop=True,
                ).then_inc(sem_proj[pb], 1)

        @block.vector
        def _(ve):
            for wi, went in enumerate(wlist):
                b = wi % NB
                pb = wi % 2
                ns = went["n_slots"]
                ve.wait_ge(sem_mm[b], wi // NB + 1)
                ve.tensor_copy(out=agg_sb[pb][:, :ns],
                               in_=agg_ps[pb][:, :ns]).then_inc(sem_agg[pb], 1)
                ve.wait_ge(sem_proj[pb], wi // 2 + 1)
                ve.tensor_copy(
                    out=stage_sb[:, went["slot0"]:went["slot0"] + ns],
                    in_=proj_ps[pb][:, :ns],
                ).then_inc(sem_stage[pb], 1)

    nc.compile()
    return nc


# ---------------------------------------------------------------------- kernel
def kernel(u_f, v_f, u_w, v_w, src, dst):
    from concourse.bass_utils import run_bass_kernel_spmd

    src = np.asarray(src)
    dst = np.asarray(dst)
    u_f = np.asarray(u_f, np.float32)
    v_f = np.asarray(v_f, np.float32)

    deg_out = np.bincount(src, minlength=N).astype(np.float32)
    deg_in = np.bincount(dst, minlength=N).astype(np.float32)
    cout = np.maximum(deg_out, 1.0) ** -0.5
    cin = np.maximum(deg_in, 1.0) ** -0.5

    sched, per_core = _build_layout(src, dst, cout, cin)

    nc = _build_nc(sched)
    u_f_b = u_f.astype(BF16)
    v_f_b = v_f.astype(BF16)
    u_w_b = np.asarray(u_w, np.float32).astype(BF16)
    v_w_b = np.asarray(v_w, np.float32).astype(BF16)
    in_maps = []
    for k in range(N_CORES):
        in_maps.append({
            "u_f": u_f_b, "v_f": v_f_b,
            "u_w": u_w_b, "v_w": v_w_b,
            "idx": per_core[k]["idx"], "sval": per_core[k]["s"],
        })
    trace = bool(os.environ.get("KERNEL_TRACE"))
    res = run_bass_kernel_spmd(nc, in_maps, core_ids=list(range(N_CORES)),
                               trace=trace)
    if trace:
        print(f"HW exec time: {res.exec_time_ns} ns")
        kernel.last_profile = res.profile_json

    out_full = np.zeros((N, D), np.float32)
    for k in range(N_CORES):
        fm = np.asarray(res.results[k]["out"], dtype=np.float32)
        rows = np.ascontiguousarray(fm.T)     # [tot_slots, 128]
        slot0 = 0
        for phase in range(2):
            dsts = per_core[k]["dsts"][phase]
            nslots = len(dsts)
            valid = dsts >= 0
            out_full[dsts[valid]] = rows[slot0:slot0 + nslots][valid]
            slot0 += nslots
    return out_full


# revision 18
# speedup vs baseline: 1.7153x; 1.0032x over previous
"""Bipartite GCN message-passing kernel for 8 Trainium2 NeuronCores.

Math (reference): rst = deg_in^-1/2 * segsum_dst( (node_f @ W_side) * deg_out^-1/2 [src] )
Refactor used here (projection is linear, graph strictly bipartite):
    rst[d] = ( sum_{e->d} c_e * f_raw[src_e] ) @ W_side(d),
    c_e = deg_out[src]^-1/2 * deg_in[dst]^-1/2  (folded on host into scatter tiles)

Sharding: dst nodes dealt round-robin (degree-sorted) to 8 cores -> identical
compile-time schedule per core (SPMD), no collectives. Per core the device:
  1. dma_gather raw bf16 feature rows by src (256B rows). The gathers are
     spread across all 4 SWDGE queues: the Q7 ucode pins each queue's
     descriptor generation to one pair of GPSIMD cores (cpu_id/2 ==
     queue_num), so 4 in-flight gathers generate descriptors on 4 core
     pairs concurrently instead of serializing on pair 0.
  2. scatter-matmul (bf16): PSUM[128f, 512slot] += M_chunk[128e,128f].T
     @ S_chunk[128e,w] where S carries c_e at (edge_row, dst_col).
  3. projection matmul with the side weight (bf16 -> fp32 PSUM).
  4. feat-major fp32 output [128, slots]; host transposes/unpermutes.
"""
import sys
import os

for _p in ("/opt/trn_rl_repo",):
    if _p not in sys.path and os.path.isdir(_p):
        sys.path.insert(0, _p)

import numpy as np
import ml_dtypes

BF16 = ml_dtypes.bfloat16

N_U = 50000
N_V = 50000
N = N_U + N_V
D = 128
E = 1600000
N_CORES = 8
HALF = 25000          # int16-safe table window
WIN = 512             # dst slots per PSUM window
P = 128
NB = 4                # M/S bank depth (gather pipeline)
NQ = 4                # SWDGE queues (each owns one GPSIMD core pair)


# ----------------------------------------------------------------- host layout
def _build_layout(src, dst, cout, cin):
    """Canonical schedule + per-core edge/scatter data.

    Returns (schedule, per_core), where schedule is compile-time (identical
    across cores) and per_core holds idx/S arrays + output dst mapping.
    """
    layout_phases = []
    per_core_idx = [[] for _ in range(N_CORES)]
    per_core_sval = [[] for _ in range(N_CORES)]   # aligned with idx positions
    per_core_dsts = [[] for _ in range(N_CORES)]   # slot -> global dst id (-1 pad)

    for phase in range(2):
        if phase == 0:       # dsts are v-nodes, sources u-side
            mask = dst >= N_U
            d_local = dst[mask] - N_U
            s_local = src[mask]
            dst_base = N_U
        else:                # dsts are u-nodes, sources v-side
            mask = dst < N_U
            d_local = dst[mask]
            s_local = src[mask] - N_U
            dst_base = 0
        half = (s_local >= HALF).astype(np.int64)
        s_half_local = s_local - half * HALF

        n_dst = N_U
        a_cnt = np.bincount(d_local[half == 0], minlength=n_dst)
        b_cnt = np.bincount(d_local[half == 1], minlength=n_dst)

        order = np.lexsort((np.arange(n_dst), b_cnt, a_cnt))
        rank = np.empty(n_dst, np.int64)
        rank[order] = np.arange(n_dst)

        slots_per_core = (n_dst + N_CORES - 1) // N_CORES  # 6250
        # canonical per-slot degrees = max over cores (clipped >= 1)
        a_mat = np.zeros((N_CORES, slots_per_core), np.int64)
        b_mat = np.zeros((N_CORES, slots_per_core), np.int64)
        dst_mat = np.full((N_CORES, slots_per_core), -1, np.int64)
        r = np.arange(n_dst)
        a_mat[r % N_CORES, r // N_CORES] = a_cnt[order]
        b_mat[r % N_CORES, r // N_CORES] = b_cnt[order]
        dst_mat[r % N_CORES, r // N_CORES] = order + dst_base
        A = np.maximum(a_mat.max(axis=0), 1)
        B = np.maximum(b_mat.max(axis=0), 1)

        for k in range(N_CORES):
            per_core_dsts[k].append(dst_mat[k])

        # ---- canonical chunking per (window, pass), no slot straddles a chunk
        n_win = (slots_per_core + WIN - 1) // WIN
        windows = []
        # canonical edge-position base per slot, per pass
        pos_base = [np.zeros(slots_per_core, np.int64) for _ in (0, 1)]
        for w in range(n_win):
            s0, s1 = w * WIN, min((w + 1) * WIN, slots_per_core)
            wininfo = {"n_slots": s1 - s0, "passes": []}
            for p_i, C in enumerate((A, B)):
                chunks = []   # (col0, w, scol0)
                cur = 0       # fill in current chunk
                cur_chunk = None
                blocks = 0
                for s in range(s0, s1):
                    c = int(C[s])
                    if cur_chunk is None or cur + c > P:
                        if cur_chunk is not None:
                            chunks.append(cur_chunk)
                        cur_chunk = {"col0": s - s0, "cols": 0}
                        blocks += 1
                        cur = 0
                    pos_base[p_i][s] = (blocks - 1) * P + cur
                    cur += c
                    cur_chunk["cols"] = (s - s0) - cur_chunk["col0"] + 1
                if cur_chunk is not None:
                    chunks.append(cur_chunk)
                wininfo["passes"].append({"chunks": chunks, "n_blocks": blocks})
            windows.append(wininfo)
        layout_phases.append({
            "n_win": n_win,
            "slots_per_core": slots_per_core,
            "windows": windows,
        })

        # ---- per-core edge placement (vectorized)
        # rank within (dst, half) group:
        grp = d_local * 2 + half
        sort_i = np.argsort(grp, kind="stable")
        grp_s = grp[sort_i]
        starts = np.r_[0, np.nonzero(np.diff(grp_s))[0] + 1]
        group_id = np.cumsum(np.r_[0, (np.diff(grp_s) != 0).astype(np.int64)])
        first_pos_of_group = starts[group_id]
        within = np.arange(len(grp_s)) - first_pos_of_group
        e_rank = np.empty(len(grp), np.int64)
        e_rank[sort_i] = within

        e_core = rank[d_local] % N_CORES
        e_slot = rank[d_local] // N_CORES
        e_win = e_slot // WIN

        # global canonical position of each edge within its (win, pass) stream:
        e_pos = np.where(half == 0,
                         pos_base[0][e_slot],
                         pos_base[1][e_slot]) + e_rank

        # canonical call sizes (blocks) per (win, pass):
        call_blocks = np.array(
            [[windows[w]["passes"][p]["n_blocks"] for p in (0, 1)]
             for w in range(n_win)], np.int64)
        # canonical flat offsets: order = win-major, pass lo then hi
        call_sizes = (call_blocks * P).reshape(-1)           # [n_win*2]
        call_off = np.r_[0, np.cumsum(call_sizes)][:-1].reshape(n_win, 2)
        tot_idx = int(call_sizes.sum())

        # canonical S layout: per chunk scol0
        s_cols_per_call = []
        for w in range(n_win):
            for p_i in (0, 1):
                ch = windows[w]["passes"][p_i]["chunks"]
                cols = np.array([c["cols"] for c in ch], np.int64)
                s_cols_per_call.append(cols)
        chunk_cols_flat = np.concatenate(s_cols_per_call)
        chunk_scol0 = np.r_[0, np.cumsum(chunk_cols_flat)][:-1]
        tot_scols = int(chunk_cols_flat.sum())
        # record scol0 / col0 back into schedule for device build
        # (scol0 made global across phases via scol_phase_base)
        scol_phase_base = sum(
            pc.shape[1] for pc in per_core_sval[0]
        ) if per_core_sval[0] else 0
        ci = 0
        for w in range(n_win):
            for p_i in (0, 1):
                for c in windows[w]["passes"][p_i]["chunks"]:
                    c["scol0"] = int(chunk_scol0[ci]) + scol_phase_base
                    ci += 1

        # per-chunk col0 arrays for edge->scol math
        chunk_col0_flat = np.concatenate(
            [np.array([c["col0"] for c in windows[w]["passes"][p_i]["chunks"]],
                      np.int64)
             for w in range(n_win) for p_i in (0, 1)])
        # chunk global id for an edge: need per-call chunk base
        chunks_per_call = np.array([len(s) for s in s_cols_per_call], np.int64)
        call_chunk_base = np.r_[0, np.cumsum(chunks_per_call)][:-1].reshape(n_win, 2)

        e_call_off = call_off[e_win, half]
        e_gpos = e_call_off + e_pos                      # global idx position
        e_chunk = call_chunk_base[e_win, half] + e_pos // P
        e_row = e_pos % P
        e_scol = chunk_scol0[e_chunk] + (e_slot - e_win * WIN) - chunk_col0_flat[e_chunk]

        e_val = (cout[s_local + (0 if phase == 0 else N_U)]
                 * cin[d_local + dst_base]).astype(np.float32)

        for k in range(N_CORES):
            m = e_core == k
            idx_flat = np.zeros(tot_idx, np.int16)
            idx_flat[e_gpos[m]] = s_half_local[m].astype(np.int16)
            sv = np.zeros((P, tot_scols), BF16)
            sv[e_row[m], e_scol[m]] = e_val[m].astype(BF16)
            per_core_idx[k].append(idx_flat)
            per_core_sval[k].append(sv)

    # wrap idx per call into the [16, n/16].T-tiled layout, concat everything
    per_core = []
    for k in range(N_CORES):
        idx_cols = []
        for phase in range(2):
            ph = layout_phases[phase]
            flat = per_core_idx[k][phase]
            off = 0
            for w in range(ph["n_win"]):
                for p_i in (0, 1):
                    nb = ph["windows"][w]["passes"][p_i]["n_blocks"]
                    n = nb * P
                    call = flat[off:off + n]
                    off += n
                    t = call.reshape(n // 16, 16).T      # [16, n/16]
                    idx_cols.append(np.tile(t, (N_CORES, 1)))
        idx_arr = np.concatenate(idx_cols, axis=1)       # [128, tot/16]
        s_arr = np.concatenate(per_core_sval[k], axis=1)  # [128, scols]
        per_core.append({"idx": idx_arr, "s": s_arr, "dsts": per_core_dsts[k]})
    return layout_phases, per_core


# ------------------------------------------------------------------ device code
def _build_nc(sched):
    import concourse.bacc as bacc
    import concourse.bass as bass
    import concourse.mybir as mybir
    from concourse._compat import get_trn_type
    from concourse.library_config import mlp

    nc = bacc.Bacc(get_trn_type() or "TRN2", target_bir_lowering=False,
                   debug=False, num_swdge_queues=NQ)
    f32 = mybir.dt.float32
    bf16 = mybir.dt.bfloat16
    u_f = nc.dram_tensor("u_f", [N_U, D], bf16, kind="ExternalInput")
    v_f = nc.dram_tensor("v_f", [N_V, D], bf16, kind="ExternalInput")
    u_w = nc.dram_tensor("u_w", [D, D], bf16, kind="ExternalInput")
    v_w = nc.dram_tensor("v_w", [D, D], bf16, kind="ExternalInput")

    # totals from schedule
    tot_idx_cols = 0
    tot_scols = 0
    tot_slots = 0
    nblk_max = 0
    for ph in sched:
        for w in ph["windows"]:
            tot_slots += w["n_slots"]
            nb = 0
            for p_i in (0, 1):
                pa = w["passes"][p_i]
                nb += pa["n_blocks"]
                tot_idx_cols += pa["n_blocks"] * P // 16
                tot_scols += sum(c["cols"] for c in pa["chunks"])
            nblk_max = max(nblk_max, nb)

    idx_in = nc.dram_tensor("idx", [P, tot_idx_cols], mybir.dt.int16,
                            kind="ExternalInput")
    s_in = nc.dram_tensor("sval", [P, tot_scols], bf16, kind="ExternalInput")
    out = nc.dram_tensor("out", [P, tot_slots], bf16, kind="ExternalOutput")

    idx_sb = nc.alloc_sbuf_tensor("idx_sb", [P, tot_idx_cols], mybir.dt.int16)
    m_sb = [nc.alloc_sbuf_tensor(f"m{i}", [P, nblk_max, P], bf16)
            for i in range(NB)]
    s_sb = [nc.alloc_sbuf_tensor(f"s{i}", [P, 2 * WIN], bf16)
            for i in range(NB)]
    agg_sb = [nc.alloc_sbuf_tensor(f"agg{i}", [P, WIN], bf16) for i in (0, 1)]
    stage_sb = nc.alloc_sbuf_tensor("stage", [P, tot_slots], bf16)
    w_sb = [nc.alloc_sbuf_tensor(f"w{i}", [P, D], bf16) for i in (0, 1)]

    agg_ps = [nc.alloc_psum_tensor(f"aps{i}", [P, WIN], f32) for i in (0, 1)]
    proj_ps = [nc.alloc_psum_tensor(f"pps{i}", [P, WIN], f32) for i in (0, 1)]

    sem_ld = nc.alloc_semaphore("ld")        # upfront loads + final store
    sem_idx = nc.alloc_semaphore("idxld")    # idx table load
    sem_s = [nc.alloc_semaphore(f"ssem{i}") for i in range(NB)]
    sem_g = [nc.alloc_semaphore(f"gsem{i}") for i in range(NB)]
    sem_mm = [nc.alloc_semaphore(f"mmsem{i}") for i in range(NB)]
    sem_agg = [nc.alloc_semaphore(f"aggsem{i}") for i in (0, 1)]
    sem_proj = [nc.alloc_semaphore(f"projsem{i}") for i in (0, 1)]
    sem_stage = [nc.alloc_semaphore(f"stsem{i}") for i in (0, 1)]

    # flatten windows across phases into one global list
    wlist = []
    icol = 0
    scol = 0
    slot0 = 0
    for phase, ph in enumerate(sched):
        for w in ph["windows"]:
            entry = {"phase": phase, "n_slots": w["n_slots"], "passes": [],
                     "slot0": slot0}
            for p_i in (0, 1):
                pa = w["passes"][p_i]
                nb = pa["n_blocks"]
                entry["passes"].append({
                    "icol": icol, "nb": nb,
                    "chunks": pa["chunks"], "scol": scol,
                })
                icol += nb * P // 16
                scol += sum(c["cols"] for c in pa["chunks"])
            slot0 += w["n_slots"]
            wlist.append(entry)
    NW = len(wlist)
    n_even = (NW + 1) // 2
    n_odd = NW // 2

    # Balanced gather sub-calls: 4 per window (one per SWDGE queue), sizes
    # as equal as possible so the 4 Q7 core pairs finish together (in-order
    # completion with ~4 outstanding Pool instructions locksteps each
    # quad on its slowest member).
    def _split(n, k):
        q, r = divmod(n, k)
        return [q + (i < r) for i in range(k)]

    for wi, went in enumerate(wlist):
        nb_lo = went["passes"][0]["nb"]
        nb_hi = went["passes"][1]["nb"]
        tot = nb_lo + nb_hi
        if nb_lo == 0:
            a = 0
        elif nb_hi == 0:
            a = 4
        else:
            a = min(3, max(1, round(4 * nb_lo / tot)))
        subs = []   # (queue, p_i, blk_s, blk_e)
        qi = 0
        for p_i, npart, nb in ((0, a, nb_lo), (1, 4 - a, nb_hi)):
            if npart == 0 or nb == 0:
                continue
            s = 0
            for sz in _split(nb, npart):
                if sz:
                    # rotate queue assignment per window so any systematic
                    # per-queue cost is spread evenly across the 4 lanes
                    subs.append(((qi + wi) % NQ, p_i, s, s + sz))
                    qi += 1
                    s += sz
        went["subs"] = subs

    # idx preload split point (after window 3) for a faster pipeline start
    idx_c0 = wlist[4]["passes"][0]["icol"] if NW > 4 else tot_idx_cols
    # output stream pieces: after these windows, flush finished stage slots
    flush_after = sorted(set([NW // 4 - 1, NW // 2 - 1, 3 * NW // 4 - 1, NW - 1]))
    flush_after = [f for f in flush_after if f >= 0]

    with nc.Block() as block:
        @block.sync
        def _(sy: bass.BassEngine):
            sy.dma_start(idx_sb[:, :idx_c0], idx_in[:, :idx_c0]).then_inc(
                sem_idx, 16)
            sy.dma_start(w_sb[0][:], u_w[:]).then_inc(sem_ld, 16)
            sy.dma_start(w_sb[1][:], v_w[:]).then_inc(sem_ld, 16)
            if idx_c0 < tot_idx_cols:
                sy.dma_start(idx_sb[:, idx_c0:], idx_in[:, idx_c0:]).then_inc(
                    sem_idx, 16)
            n_flush = 0
            flush_slot0 = 0
            for wi, went in enumerate(wlist):
                b = wi % NB
                # WAR: S bank b free after window wi-NB's matmuls done
                if wi >= NB:
                    sy.wait_ge(sem_mm[b], wi // NB)
                p0, p1 = went["passes"]
                ncols = (sum(c["cols"] for c in p0["chunks"])
                         + sum(c["cols"] for c in p1["chunks"]))
                sy.dma_start(
                    s_sb[b][:, :ncols], s_in[:, p0["scol"]:p0["scol"] + ncols]
                ).then_inc(sem_s[b], 16)
                if wi in flush_after:
                    # stream finished stage slots out while later windows run
                    sy.wait_ge(sem_stage[0], wi // 2 + 1)
                    sy.wait_ge(sem_stage[1], (wi + 1) // 2)
                    end = went["slot0"] + went["n_slots"]
                    sy.dma_start(
                        out[:, flush_slot0:end], stage_sb[:, flush_slot0:end]
                    ).then_inc(sem_ld, 16)
                    flush_slot0 = end
                    n_flush += 1
            sy.wait_ge(sem_ld, 32 + 16 * n_flush)

        @block.gpsimd
        def _(gp: bass.BassGpSimd):
            gp.load_library(mlp)
            for wi, went in enumerate(wlist):
                b = wi % NB
                phase = went["phase"]
                if phase == 0:
                    tab_lo, tab_hi = u_f[0:HALF, :], u_f[HALF:N_U, :]
                else:
                    tab_lo, tab_hi = v_f[0:HALF, :], v_f[HALF:N_V, :]
                nb_lo = went["passes"][0]["nb"]
                if wi == 0:
                    gp.wait_ge(sem_idx, 16)
                if wi == 4 and idx_c0 < tot_idx_cols:
                    gp.wait_ge(sem_idx, 32)
                if wi >= NB:
                    gp.wait_ge(sem_mm[b], wi // NB)
                for q, p_i, blk_s, blk_e in went["subs"]:
                    pa = went["passes"][p_i]
                    tab = tab_lo if p_i == 0 else tab_hi
                    nsub = (blk_e - blk_s) * P
                    blk_base = (0 if p_i == 0 else nb_lo) + blk_s
                    icol_s = pa["icol"] + blk_s * P // 16
                    gp.dma_gather(
                        m_sb[b][:, blk_base:blk_base + (blk_e - blk_s), :],
                        tab,
                        idx_sb[:, icol_s:icol_s + nsub // 16],
                        nsub, nsub, D,
                        single_packet=False,
                        queue_num=q,
                    ).then_inc(sem_g[b], 16)

        @block.tensor
        def _(te):
            g_seen = [0] * NB
            s_seen = [0] * NB
            for wi, went in enumerate(wlist):
                b = wi % NB
                pb = wi % 2
                phase = went["phase"]
                # all gather sub-calls + S stream for this window
                g_seen[b] += 16 * len(went["subs"])
                s_seen[b] += 16
                te.wait_ge(sem_g[b], g_seen[b])
                te.wait_ge(sem_s[b], s_seen[b])
                # agg_ps WAR vs vector copy of window wi-2
                if wi >= 2:
                    te.wait_ge(sem_agg[pb], wi // 2)
                ns = went["n_slots"]
                blk0 = 0
                last = None
                for p_i in (0, 1):
                    pa = went["passes"][p_i]
                    for ci, ch in enumerate(pa["chunks"]):
                        last = (p_i, ci)
                first = True
                for p_i in (0, 1):
                    pa = went["passes"][p_i]
                    for ci, ch in enumerate(pa["chunks"]):
                        blk = blk0 + ci
                        sc = ch["scol0"] - went["passes"][0]["scol"]
                        mm = te.matmul(
                            out=agg_ps[pb][:, ch["col0"]:ch["col0"] + ch["cols"]],
                            lhsT=m_sb[b][:, blk, :],
                            rhs=s_sb[b][:, sc:sc + ch["cols"]],
                            start=first,
                            stop=((p_i, ci) == last),
                        )
                        first = False
                        if (p_i, ci) == last:
                            mm.then_inc(sem_mm[b], 1)
                    blk0 += pa["nb"]
                # projection: wait for vector to copy agg->sbuf (this window)
                te.wait_ge(sem_agg[pb], wi // 2 + 1)
                # proj_ps WAR vs vector stage copy of window wi-2
                if wi >= 2:
                    te.wait_ge(sem_stage[pb], wi // 2)
                te.matmul(
                    out=proj_ps[pb][:, :ns],
                    lhsT=w_sb[phase][:],
                    rhs=agg_sb[pb][:, :ns],
                    start=True, stop=True,
                ).then_inc(sem_proj[pb], 1)

        @block.vector
        def _(ve):
            for wi, went in enumerate(wlist):
                b = wi % NB
                pb = wi % 2
                ns = went["n_slots"]
                ve.wait_ge(sem_mm[b], wi // NB + 1)
                ve.tensor_copy(out=agg_sb[pb][:, :ns],
                               in_=agg_ps[pb][:, :ns]).then_inc(sem_agg[pb], 1)
                ve.wait_ge(sem_proj[pb], wi // 2 + 1)
                ve.tensor_copy(
                    out=stage_sb[:, went["slot0"]:went["slot0"] + ns],
                    in_=proj_ps[pb][:, :ns],
                ).then_inc(sem_stage[pb], 1)

    nc.compile()
    return nc


# ---------------------------------------------------------------------- kernel
def kernel(u_f, v_f, u_w, v_w, src, dst):
    from concourse.bass_utils import run_bass_kernel_spmd

    src = np.asarray(src)
    dst = np.asarray(dst)
    u_f = np.asarray(u_f, np.float32)
    v_f = np.asarray(v_f, np.float32)

    deg_out = np.bincount(src, minlength=N).astype(np.float32)
    deg_in = np.bincount(dst, minlength=N).astype(np.float32)
    cout = np.maximum(deg_out, 1.0) ** -0.5
    cin = np.maximum(deg_in, 1.0) ** -0.5

    sched, per_core = _build_layout(src, dst, cout, cin)

    nc = _build_nc(sched)
    u_f_b = u_f.astype(BF16)
    v_f_b = v_f.astype(BF16)
    u_w_b = np.asarray(u_w, np.float32).astype(BF16)
    v_w_b = np.asarray(v_w, np.float32).astype(BF16)
    in_maps = []
    for k in range(N_CORES):
        in_maps.append({
            "u_f": u_f_b, "v_f": v_f_b,
            "u_w": u_w_b, "v_w": v_w_b,
            "idx": per_core[k]["idx"], "sval": per_core[k]["s"],
        })
    trace = bool(os.environ.get("KERNEL_TRACE"))
    res = run_bass_kernel_spmd(nc, in_maps, core_ids=list(range(N_CORES)),
                               trace=trace)
    if trace:
        print(f"HW exec time: {res.exec_time_ns} ns")
        kernel.last_profile = res.profile_json

    out_full = np.zeros((N, D), np.float32)
    for k in range(N_CORES):
        fm = np.asarray(res.results[k]["out"], dtype=np.float32)
        rows = np.ascontiguousarray(fm.T)     # [tot_slots, 128]
        slot0 = 0
        for phase in range(2):
            dsts = per_core[k]["dsts"][phase]
            nslots = len(dsts)
            valid = dsts >= 0
            out_full[dsts[valid]] = rows[slot0:slot0 + nslots][valid]
            slot0 += nslots
    return out_full


# revision 19
# speedup vs baseline: 1.7191x; 1.0022x over previous
"""Bipartite GCN message-passing kernel for 8 Trainium2 NeuronCores.

Math (reference): rst = deg_in^-1/2 * segsum_dst( (node_f @ W_side) * deg_out^-1/2 [src] )
Refactor used here (projection is linear, graph strictly bipartite):
    rst[d] = ( sum_{e->d} c_e * f_raw[src_e] ) @ W_side(d),
    c_e = deg_out[src]^-1/2 * deg_in[dst]^-1/2  (folded on host into scatter tiles)

Sharding: dst nodes dealt round-robin (degree-sorted) to 8 cores -> identical
compile-time schedule per core (SPMD), no collectives. Per core the device:
  1. dma_gather raw bf16 feature rows by src (256B rows). The gathers are
     spread across all 4 SWDGE queues: the Q7 ucode pins each queue's
     descriptor generation to one pair of GPSIMD cores (cpu_id/2 ==
     queue_num), so 4 in-flight gathers generate descriptors on 4 core
     pairs concurrently instead of serializing on pair 0.
  2. scatter-matmul (bf16): PSUM[128f, 512slot] += M_chunk[128e,128f].T
     @ S_chunk[128e,w] where S carries c_e at (edge_row, dst_col).
  3. projection matmul with the side weight (bf16 -> fp32 PSUM).
  4. feat-major fp32 output [128, slots]; host transposes/unpermutes.
"""
import sys
import os

for _p in ("/opt/trn_rl_repo",):
    if _p not in sys.path and os.path.isdir(_p):
        sys.path.insert(0, _p)

import numpy as np
import ml_dtypes

BF16 = ml_dtypes.bfloat16

N_U = 50000
N_V = 50000
N = N_U + N_V
D = 128
E = 1600000
N_CORES = 8
HALF = 25000          # int16-safe table window
WIN = 512             # dst slots per PSUM window
P = 128
NB = 4                # M/S bank depth (gather pipeline)
NQ = 4                # SWDGE queues (each owns one GPSIMD core pair)


# ----------------------------------------------------------------- host layout
def _build_layout(src, dst, cout, cin):
    """Canonical schedule + per-core edge/scatter data.

    Returns (schedule, per_core), where schedule is compile-time (identical
    across cores) and per_core holds idx/S arrays + output dst mapping.
    """
    layout_phases = []
    per_core_idx = [[] for _ in range(N_CORES)]
    per_core_sval = [[] for _ in range(N_CORES)]   # aligned with idx positions
    per_core_dsts = [[] for _ in range(N_CORES)]   # slot -> global dst id (-1 pad)

    for phase in range(2):
        if phase == 0:       # dsts are v-nodes, sources u-side
            mask = dst >= N_U
            d_local = dst[mask] - N_U
            s_local = src[mask]
            dst_base = N_U
        else:                # dsts are u-nodes, sources v-side
            mask = dst < N_U
            d_local = dst[mask]
            s_local = src[mask] - N_U
            dst_base = 0
        half = (s_local >= HALF).astype(np.int64)
        s_half_local = s_local - half * HALF

        n_dst = N_U
        a_cnt = np.bincount(d_local[half == 0], minlength=n_dst)
        b_cnt = np.bincount(d_local[half == 1], minlength=n_dst)

        order = np.lexsort((np.arange(n_dst), b_cnt, a_cnt))
        rank = np.empty(n_dst, np.int64)
        rank[order] = np.arange(n_dst)

        slots_per_core = (n_dst + N_CORES - 1) // N_CORES  # 6250
        # canonical per-slot degrees = max over cores (clipped >= 1)
        a_mat = np.zeros((N_CORES, slots_per_core), np.int64)
        b_mat = np.zeros((N_CORES, slots_per_core), np.int64)
        dst_mat = np.full((N_CORES, slots_per_core), -1, np.int64)
        r = np.arange(n_dst)
        a_mat[r % N_CORES, r // N_CORES] = a_cnt[order]
        b_mat[r % N_CORES, r // N_CORES] = b_cnt[order]
        dst_mat[r % N_CORES, r // N_CORES] = order + dst_base
        A = np.maximum(a_mat.max(axis=0), 1)
        B = np.maximum(b_mat.max(axis=0), 1)

        for k in range(N_CORES):
            per_core_dsts[k].append(dst_mat[k])

        # ---- canonical chunking per (window, pass), no slot straddles a chunk
        n_win = (slots_per_core + WIN - 1) // WIN
        windows = []
        # canonical edge-position base per slot, per pass
        pos_base = [np.zeros(slots_per_core, np.int64) for _ in (0, 1)]
        for w in range(n_win):
            s0, s1 = w * WIN, min((w + 1) * WIN, slots_per_core)
            wininfo = {"n_slots": s1 - s0, "passes": []}
            for p_i, C in enumerate((A, B)):
                chunks = []   # (col0, w, scol0)
                cur = 0       # fill in current chunk
                cur_chunk = None
                blocks = 0
                for s in range(s0, s1):
                    c = int(C[s])
                    if cur_chunk is None or cur + c > P:
                        if cur_chunk is not None:
                            chunks.append(cur_chunk)
                        cur_chunk = {"col0": s - s0, "cols": 0}
                        blocks += 1
                        cur = 0
                    pos_base[p_i][s] = (blocks - 1) * P + cur
                    cur += c
                    cur_chunk["cols"] = (s - s0) - cur_chunk["col0"] + 1
                if cur_chunk is not None:
                    chunks.append(cur_chunk)
                wininfo["passes"].append({"chunks": chunks, "n_blocks": blocks})
            windows.append(wininfo)
        layout_phases.append({
            "n_win": n_win,
            "slots_per_core": slots_per_core,
            "windows": windows,
        })

        # ---- per-core edge placement (vectorized)
        # rank within (dst, half) group:
        grp = d_local * 2 + half
        sort_i = np.argsort(grp, kind="stable")
        grp_s = grp[sort_i]
        starts = np.r_[0, np.nonzero(np.diff(grp_s))[0] + 1]
        group_id = np.cumsum(np.r_[0, (np.diff(grp_s) != 0).astype(np.int64)])
        first_pos_of_group = starts[group_id]
        within = np.arange(len(grp_s)) - first_pos_of_group
        e_rank = np.empty(len(grp), np.int64)
        e_rank[sort_i] = within

        e_core = rank[d_local] % N_CORES
        e_slot = rank[d_local] // N_CORES
        e_win = e_slot // WIN

        # global canonical position of each edge within its (win, pass) stream:
        e_pos = np.where(half == 0,
                         pos_base[0][e_slot],
                         pos_base[1][e_slot]) + e_rank

        # canonical call sizes (blocks) per (win, pass):
        call_blocks = np.array(
            [[windows[w]["passes"][p]["n_blocks"] for p in (0, 1)]
             for w in range(n_win)], np.int64)
        # canonical flat offsets: order = win-major, pass lo then hi
        call_sizes = (call_blocks * P).reshape(-1)           # [n_win*2]
        call_off = np.r_[0, np.cumsum(call_sizes)][:-1].reshape(n_win, 2)
        tot_idx = int(call_sizes.sum())

        # canonical S layout: per chunk scol0
        s_cols_per_call = []
        for w in range(n_win):
            for p_i in (0, 1):
                ch = windows[w]["passes"][p_i]["chunks"]
                cols = np.array([c["cols"] for c in ch], np.int64)
                s_cols_per_call.append(cols)
        chunk_cols_flat = np.concatenate(s_cols_per_call)
        chunk_scol0 = np.r_[0, np.cumsum(chunk_cols_flat)][:-1]
        tot_scols = int(chunk_cols_flat.sum())
        # record scol0 / col0 back into schedule for device build
        # (scol0 made global across phases via scol_phase_base)
        scol_phase_base = sum(
            pc.shape[1] for pc in per_core_sval[0]
        ) if per_core_sval[0] else 0
        ci = 0
        for w in range(n_win):
            for p_i in (0, 1):
                for c in windows[w]["passes"][p_i]["chunks"]:
                    c["scol0"] = int(chunk_scol0[ci]) + scol_phase_base
                    ci += 1

        # per-chunk col0 arrays for edge->scol math
        chunk_col0_flat = np.concatenate(
            [np.array([c["col0"] for c in windows[w]["passes"][p_i]["chunks"]],
                      np.int64)
             for w in range(n_win) for p_i in (0, 1)])
        # chunk global id for an edge: need per-call chunk base
        chunks_per_call = np.array([len(s) for s in s_cols_per_call], np.int64)
        call_chunk_base = np.r_[0, np.cumsum(chunks_per_call)][:-1].reshape(n_win, 2)

        e_call_off = call_off[e_win, half]
        e_gpos = e_call_off + e_pos                      # global idx position
        e_chunk = call_chunk_base[e_win, half] + e_pos // P
        e_row = e_pos % P
        e_scol = chunk_scol0[e_chunk] + (e_slot - e_win * WIN) - chunk_col0_flat[e_chunk]

        e_val = (cout[s_local + (0 if phase == 0 else N_U)]
                 * cin[d_local + dst_base]).astype(np.float32)

        for k in range(N_CORES):
            m = e_core == k
            idx_flat = np.zeros(tot_idx, np.int16)
            idx_flat[e_gpos[m]] = s_half_local[m].astype(np.int16)
            sv = np.zeros((P, tot_scols), BF16)
            sv[e_row[m], e_scol[m]] = e_val[m].astype(BF16)
            per_core_idx[k].append(idx_flat)
            per_core_sval[k].append(sv)

    # wrap idx per call into the [16, n/16].T-tiled layout, concat everything
    per_core = []
    for k in range(N_CORES):
        idx_cols = []
        for phase in range(2):
            ph = layout_phases[phase]
            flat = per_core_idx[k][phase]
            off = 0
            for w in range(ph["n_win"]):
                for p_i in (0, 1):
                    nb = ph["windows"][w]["passes"][p_i]["n_blocks"]
                    n = nb * P
                    call = flat[off:off + n]
                    off += n
                    t = call.reshape(n // 16, 16).T      # [16, n/16]
                    idx_cols.append(np.tile(t, (N_CORES, 1)))
        idx_arr = np.concatenate(idx_cols, axis=1)       # [128, tot/16]
        s_arr = np.concatenate(per_core_sval[k], axis=1)  # [128, scols]
        per_core.append({"idx": idx_arr, "s": s_arr, "dsts": per_core_dsts[k]})
    return layout_phases, per_core


# ------------------------------------------------------------------ device code
def _build_nc(sched):
    import concourse.bacc as bacc
    import concourse.bass as bass
    import concourse.mybir as mybir
    from concourse._compat import get_trn_type
    from concourse.library_config import mlp

    nc = bacc.Bacc(get_trn_type() or "TRN2", target_bir_lowering=False,
                   debug=False, num_swdge_queues=NQ)
    f32 = mybir.dt.float32
    bf16 = mybir.dt.bfloat16
    u_f = nc.dram_tensor("u_f", [N_U, D], bf16, kind="ExternalInput")
    v_f = nc.dram_tensor("v_f", [N_V, D], bf16, kind="ExternalInput")
    u_w = nc.dram_tensor("u_w", [D, D], bf16, kind="ExternalInput")
    v_w = nc.dram_tensor("v_w", [D, D], bf16, kind="ExternalInput")

    # totals from schedule
    tot_idx_cols = 0
    tot_scols = 0
    tot_slots = 0
    nblk_max = 0
    for ph in sched:
        for w in ph["windows"]:
            tot_slots += w["n_slots"]
            nb = 0
            for p_i in (0, 1):
                pa = w["passes"][p_i]
                nb += pa["n_blocks"]
                tot_idx_cols += pa["n_blocks"] * P // 16
                tot_scols += sum(c["cols"] for c in pa["chunks"])
            nblk_max = max(nblk_max, nb)

    idx_in = nc.dram_tensor("idx", [P, tot_idx_cols], mybir.dt.int16,
                            kind="ExternalInput")
    s_in = nc.dram_tensor("sval", [P, tot_scols], bf16, kind="ExternalInput")
    out = nc.dram_tensor("out", [P, tot_slots], bf16, kind="ExternalOutput")

    idx_sb = nc.alloc_sbuf_tensor("idx_sb", [P, tot_idx_cols], mybir.dt.int16)
    m_sb = [nc.alloc_sbuf_tensor(f"m{i}", [P, nblk_max, P], bf16)
            for i in range(NB)]
    s_sb = [nc.alloc_sbuf_tensor(f"s{i}", [P, 2 * WIN], bf16)
            for i in range(NB)]
    agg_sb = [nc.alloc_sbuf_tensor(f"agg{i}", [P, WIN], bf16) for i in (0, 1)]
    stage_sb = nc.alloc_sbuf_tensor("stage", [P, tot_slots], bf16)
    w_sb = [nc.alloc_sbuf_tensor(f"w{i}", [P, D], bf16) for i in (0, 1)]

    agg_ps = [nc.alloc_psum_tensor(f"aps{i}", [P, WIN], f32) for i in (0, 1)]
    proj_ps = [nc.alloc_psum_tensor(f"pps{i}", [P, WIN], f32) for i in (0, 1)]

    sem_ld = nc.alloc_semaphore("ld")        # upfront loads + final store
    sem_idx = nc.alloc_semaphore("idxld")    # idx table load
    sem_s = [nc.alloc_semaphore(f"ssem{i}") for i in range(NB)]
    sem_g = [nc.alloc_semaphore(f"gsem{i}") for i in range(NB)]
    sem_mm = [nc.alloc_semaphore(f"mmsem{i}") for i in range(NB)]
    sem_agg = [nc.alloc_semaphore(f"aggsem{i}") for i in (0, 1)]
    sem_proj = [nc.alloc_semaphore(f"projsem{i}") for i in (0, 1)]
    sem_stage = [nc.alloc_semaphore(f"stsem{i}") for i in (0, 1)]

    # flatten windows across phases into one global list
    wlist = []
    icol = 0
    scol = 0
    slot0 = 0
    for phase, ph in enumerate(sched):
        for w in ph["windows"]:
            entry = {"phase": phase, "n_slots": w["n_slots"], "passes": [],
                     "slot0": slot0}
            for p_i in (0, 1):
                pa = w["passes"][p_i]
                nb = pa["n_blocks"]
                entry["passes"].append({
                    "icol": icol, "nb": nb,
                    "chunks": pa["chunks"], "scol": scol,
                })
                icol += nb * P // 16
                scol += sum(c["cols"] for c in pa["chunks"])
            slot0 += w["n_slots"]
            wlist.append(entry)
    NW = len(wlist)
    n_even = (NW + 1) // 2
    n_odd = NW // 2

    # Balanced gather sub-calls: 4 per window (one per SWDGE queue), sizes
    # as equal as possible so the 4 Q7 core pairs finish together (in-order
    # completion with ~4 outstanding Pool instructions locksteps each
    # quad on its slowest member).
    def _split(n, k):
        q, r = divmod(n, k)
        return [q + (i < r) for i in range(k)]

    for wi, went in enumerate(wlist):
        nb_lo = went["passes"][0]["nb"]
        nb_hi = went["passes"][1]["nb"]
        tot = nb_lo + nb_hi
        if nb_lo == 0:
            a = 0
        elif nb_hi == 0:
            a = 4
        else:
            a = min(3, max(1, round(4 * nb_lo / tot)))
        subs = []   # (queue, p_i, blk_s, blk_e)
        qi = 0
        for p_i, npart, nb in ((0, a, nb_lo), (1, 4 - a, nb_hi)):
            if npart == 0 or nb == 0:
                continue
            s = 0
            for sz in _split(nb, npart):
                if sz:
                    # rotate queue assignment per window so any systematic
                    # per-queue cost is spread evenly across the 4 lanes
                    subs.append(((qi + wi) % NQ, p_i, s, s + sz))
                    qi += 1
                    s += sz
        # emit smallest first: under in-order retirement the first emitted
        # call retires fastest, freeing an exec-queue slot for the next
        # window's gathers sooner (queues stay attached to their pieces)
        subs.sort(key=lambda t: t[3] - t[2])
        went["subs"] = subs

    # idx preload split point (after window 3) for a faster pipeline start
    idx_c0 = wlist[4]["passes"][0]["icol"] if NW > 4 else tot_idx_cols
    # output stream pieces: after these windows, flush finished stage slots
    flush_after = sorted(set([NW // 4 - 1, NW // 2 - 1, 3 * NW // 4 - 1, NW - 1]))
    flush_after = [f for f in flush_after if f >= 0]

    with nc.Block() as block:
        @block.sync
        def _(sy: bass.BassEngine):
            sy.dma_start(idx_sb[:, :idx_c0], idx_in[:, :idx_c0]).then_inc(
                sem_idx, 16)
            sy.dma_start(w_sb[0][:], u_w[:]).then_inc(sem_ld, 16)
            sy.dma_start(w_sb[1][:], v_w[:]).then_inc(sem_ld, 16)
            if idx_c0 < tot_idx_cols:
                sy.dma_start(idx_sb[:, idx_c0:], idx_in[:, idx_c0:]).then_inc(
                    sem_idx, 16)
            n_flush = 0
            flush_slot0 = 0
            for wi, went in enumerate(wlist):
                b = wi % NB
                # WAR: S bank b free after window wi-NB's matmuls done
                if wi >= NB:
                    sy.wait_ge(sem_mm[b], wi // NB)
                p0, p1 = went["passes"]
                ncols = (sum(c["cols"] for c in p0["chunks"])
                         + sum(c["cols"] for c in p1["chunks"]))
                sy.dma_start(
                    s_sb[b][:, :ncols], s_in[:, p0["scol"]:p0["scol"] + ncols]
                ).then_inc(sem_s[b], 16)
                if wi in flush_after:
                    # stream finished stage slots out while later windows run
                    sy.wait_ge(sem_stage[0], wi // 2 + 1)
                    sy.wait_ge(sem_stage[1], (wi + 1) // 2)
                    end = went["slot0"] + went["n_slots"]
                    sy.dma_start(
                        out[:, flush_slot0:end], stage_sb[:, flush_slot0:end]
                    ).then_inc(sem_ld, 16)
                    flush_slot0 = end
                    n_flush += 1
            sy.wait_ge(sem_ld, 32 + 16 * n_flush)

        @block.gpsimd
        def _(gp: bass.BassGpSimd):
            gp.load_library(mlp)
            for wi, went in enumerate(wlist):
                b = wi % NB
                phase = went["phase"]
                if phase == 0:
                    tab_lo, tab_hi = u_f[0:HALF, :], u_f[HALF:N_U, :]
                else:
                    tab_lo, tab_hi = v_f[0:HALF, :], v_f[HALF:N_V, :]
                nb_lo = went["passes"][0]["nb"]
                if wi == 0:
                    gp.wait_ge(sem_idx, 16)
                if wi == 4 and idx_c0 < tot_idx_cols:
                    gp.wait_ge(sem_idx, 32)
                if wi >= NB:
                    gp.wait_ge(sem_mm[b], wi // NB)
                for q, p_i, blk_s, blk_e in went["subs"]:
                    pa = went["passes"][p_i]
                    tab = tab_lo if p_i == 0 else tab_hi
                    nsub = (blk_e - blk_s) * P
                    blk_base = (0 if p_i == 0 else nb_lo) + blk_s
                    icol_s = pa["icol"] + blk_s * P // 16
                    gp.dma_gather(
                        m_sb[b][:, blk_base:blk_base + (blk_e - blk_s), :],
                        tab,
                        idx_sb[:, icol_s:icol_s + nsub // 16],
                        nsub, nsub, D,
                        single_packet=False,
                        queue_num=q,
                    ).then_inc(sem_g[b], 16)

        @block.tensor
        def _(te):
            g_seen = [0] * NB
            s_seen = [0] * NB
            for wi, went in enumerate(wlist):
                b = wi % NB
                pb = wi % 2
                phase = went["phase"]
                # all gather sub-calls + S stream for this window
                g_seen[b] += 16 * len(went["subs"])
                s_seen[b] += 16
                te.wait_ge(sem_g[b], g_seen[b])
                te.wait_ge(sem_s[b], s_seen[b])
                # agg_ps WAR vs vector copy of window wi-2
                if wi >= 2:
                    te.wait_ge(sem_agg[pb], wi // 2)
                ns = went["n_slots"]
                blk0 = 0
                last = None
                for p_i in (0, 1):
                    pa = went["passes"][p_i]
                    for ci, ch in enumerate(pa["chunks"]):
                        last = (p_i, ci)
                first = True
                for p_i in (0, 1):
                    pa = went["passes"][p_i]
                    for ci, ch in enumerate(pa["chunks"]):
                        blk = blk0 + ci
                        sc = ch["scol0"] - went["passes"][0]["scol"]
                        mm = te.matmul(
                            out=agg_ps[pb][:, ch["col0"]:ch["col0"] + ch["cols"]],
                            lhsT=m_sb[b][:, blk, :],
                            rhs=s_sb[b][:, sc:sc + ch["cols"]],
                            start=first,
                            stop=((p_i, ci) == last),
                        )
                        first = False
                        if (p_i, ci) == last:
                            mm.then_inc(sem_mm[b], 1)
                    blk0 += pa["nb"]
                # projection: wait for vector to copy agg->sbuf (this window)
                te.wait_ge(sem_agg[pb], wi // 2 + 1)
                # proj_ps WAR vs vector stage copy of window wi-2
                if wi >= 2:
                    te.wait_ge(sem_stage[pb], wi // 2)
                te.matmul(
                    out=proj_ps[pb][:, :ns],
                    lhsT=w_sb[phase][:],
                    rhs=agg_sb[pb][:, :ns],
                    start=True, stop=True,
                ).then_inc(sem_proj[pb], 1)

        @block.vector
        def _(ve):
            for wi, went in enumerate(wlist):
                b = wi % NB
                pb = wi % 2
                ns = went["n_slots"]
                ve.wait_ge(sem_mm[b], wi // NB + 1)
                ve.tensor_copy(out=agg_sb[pb][:, :ns],
                               in_=agg_ps[pb][:, :ns]).then_inc(sem_agg[pb], 1)
                ve.wait_ge(sem_proj[pb], wi // 2 + 1)
                ve.tensor_copy(
                    out=stage_sb[:, went["slot0"]:went["slot0"] + ns],
                    in_=proj_ps[pb][:, :ns],
                ).then_inc(sem_stage[pb], 1)

    nc.compile()
    return nc


# ---------------------------------------------------------------------- kernel
def kernel(u_f, v_f, u_w, v_w, src, dst):
    from concourse.bass_utils import run_bass_kernel_spmd

    src = np.asarray(src)
    dst = np.asarray(dst)
    u_f = np.asarray(u_f, np.float32)
    v_f = np.asarray(v_f, np.float32)

    deg_out = np.bincount(src, minlength=N).astype(np.float32)
    deg_in = np.bincount(dst, minlength=N).astype(np.float32)
    cout = np.maximum(deg_out, 1.0) ** -0.5
    cin = np.maximum(deg_in, 1.0) ** -0.5

    sched, per_core = _build_layout(src, dst, cout, cin)

    nc = _build_nc(sched)
    u_f_b = u_f.astype(BF16)
    v_f_b = v_f.astype(BF16)
    u_w_b = np.asarray(u_w, np.float32).astype(BF16)
    v_w_b = np.asarray(v_w, np.float32).astype(BF16)
    in_maps = []
    for k in range(N_CORES):
        in_maps.append({
            "u_f": u_f_b, "v_f": v_f_b,
            "u_w": u_w_b, "v_w": v_w_b,
            "idx": per_core[k]["idx"], "sval": per_core[k]["s"],
        })
    trace = bool(os.environ.get("KERNEL_TRACE"))
    res = run_bass_kernel_spmd(nc, in_maps, core_ids=list(range(N_CORES)),
                               trace=trace)
    if trace:
        print(f"HW exec time: {res.exec_time_ns} ns")
        kernel.last_profile = res.profile_json

    out_full = np.zeros((N, D), np.float32)
    for k in range(N_CORES):
        fm = np.asarray(res.results[k]["out"], dtype=np.float32)
        rows = np.ascontiguousarray(fm.T)     # [tot_slots, 128]
        slot0 = 0
        for phase in range(2):
            dsts = per_core[k]["dsts"][phase]
            nslots = len(dsts)
            valid = dsts >= 0
            out_full[dsts[valid]] = rows[slot0:slot0 + nslots][valid]
            slot0 += nslots
    return out_full
